# revision 1
# baseline (speedup 1.0000x reference)
"""Windowed-attention ViT block (SAM-style) on 8 TRN2 NeuronCores.

Feature-major ("^T") layout [dim, tokens] on device. Per core: 13 window
slots (8 interior + 2 right-edge + 2 bottom-edge + 1 corner-or-dummy) so
all 8 cores run one identical SPMD program. Host does window partition /
packing (pure data movement) and folds LN scales into adjacent weights;
device does all FLOPs (bf16 matmuls, f32 psum, f32 residual path).
Rel-pos bias applied multiplicatively: exp(S+B) = exp(S)*EC1*EC2, with EC
gathered from exp(q @ relpos_rev) via per-row Toeplitz-strided DMAs.
"""
import numpy as np
import ml_dtypes
from contextlib import ExitStack

DIM = 768
NH = 12
HD = 64
WS = 14
H = W = 64
MLP_H = 3072
EPS = 1e-5
SCALE = HD ** -0.5
B = 4
NCORES = 8
TW = WS * WS  # 196
NW_CORE = 13
CORE_TOK = NW_CORE * TW  # 2548

WCLASSES = [(14, 14)] * 8 + [(14, 8)] * 2 + [(8, 14)] * 2 + [(8, 8)]
MLP_GROUPS = [(0, 1), (2, 3), (4, 5), (6, 7), (8, 9), (10, 11), (12,)]

BF16 = ml_dtypes.bfloat16


def token_order(rh, rw):
    order = [(r, c) for r in range(rh) for c in range(rw)]
    order += [(r, c) for r in range(rh) for c in range(rw, WS)]
    order += [(r, c) for r in range(rh, WS) for c in range(WS)]
    return order


def attn_chunks(rh, rw):
    """Token-offset chunks of the real block: (t0, csz); row-aligned."""
    if rh * rw == 196:
        return [(0, 98), (98, 98)]
    return [(0, rh * rw)]


def col_groups(rh, rw):
    """Key-column groups: (start, n_r2, r2_0, n_c2, c2_0)."""
    g = [(0, rh, 0, rw, 0)]
    if rw < WS:
        g.append((rh * rw, rh, 0, WS - rw, rw))
    if rh < WS:
        g.append((rh * WS, WS - rh, rh, WS, 0))
    return g


_CACHE = {}


def _build(n_windows=NW_CORE, classes=None, groups=None, exact_gelu=True):
    import concourse.bass as bass
    import concourse.mybir as mybir
    import concourse.tile as tile
    from concourse import bacc
    from concourse.masks import make_identity

    F32 = mybir.dt.float32
    BF = mybir.dt.bfloat16
    AF = mybir.ActivationFunctionType
    AX = mybir.AxisListType

    if classes is None:
        classes = WCLASSES[:n_windows]
    if groups is None:
        groups = [tuple(s for s in g if s < n_windows) for g in MLP_GROUPS]
        groups = [g for g in groups if g]
    ncols = n_windows * TW

    nc = bacc.Bacc("TRN2", target_bir_lowering=False, debug=False,
                   enable_asserts=False, num_devices=NCORES)

    xwT_d = nc.dram_tensor("xwT", [DIM, ncols], F32, kind="ExternalInput")
    wqkv_d = nc.dram_tensor("wqkv", [6, 128, 3 * DIM], BF, kind="ExternalInput")
    bqkv_d = nc.dram_tensor("bqkv", [128, 18], F32, kind="ExternalInput")
    relhw_d = nc.dram_tensor("relhw", [128, 2, 27], BF, kind="ExternalInput")
    wproj_d = nc.dram_tensor("wproj", [6, 128, DIM], BF, kind="ExternalInput")
    bproj_d = nc.dram_tensor("bproj", [1, DIM], BF, kind="ExternalInput")
    w1_d = nc.dram_tensor("w1", [6, 128, MLP_H], BF, kind="ExternalInput")
    b1_d = nc.dram_tensor("b1", [128, 24], F32, kind="ExternalInput")
    w2_d = nc.dram_tensor("w2", [24, 128, DIM], BF, kind="ExternalInput")
    b2_d = nc.dram_tensor("b2", [1, DIM], BF, kind="ExternalInput")
    out_d = nc.dram_tensor("outT", [DIM, ncols], F32, kind="ExternalOutput")

    with tile.TileContext(nc) as tc, ExitStack() as ctx:
        wp = ctx.enter_context(tc.tile_pool(name="weights", bufs=1))
        sb = ctx.enter_context(tc.tile_pool(name="sb", bufs=2))
        sb1 = ctx.enter_context(tc.tile_pool(name="sb1", bufs=1))
        sb2 = ctx.enter_context(tc.tile_pool(name="sb2", bufs=2))
        ps_mm = ctx.enter_context(tc.tile_pool(name="ps_mm", bufs=3, space="PSUM"))
        ps_av = ctx.enter_context(tc.tile_pool(name="ps_av", bufs=1, space="PSUM"))
        ps_bc = ctx.enter_context(tc.tile_pool(name="ps_bc", bufs=1, space="PSUM"))
        ps_st = ctx.enter_context(tc.tile_pool(name="ps_st", bufs=1, space="PSUM"))
        ps_tr = ctx.enter_context(tc.tile_pool(name="ps_tr", bufs=2, space="PSUM"))
        dr = ctx.enter_context(tc.tile_pool(name="dr", bufs=2, space="DRAM"))

        # ---- constants ----
        wqkv = wp.tile([128, 6, 3 * DIM], BF)
        wproj = wp.tile([128, 6, DIM], BF)
        w1 = wp.tile([128, 6, MLP_H], BF)
        w2 = wp.tile([128, 24, DIM], BF)
        # wqkv on the sync queue (needed first, ahead of window-0 xw);
        # the bulky later-phase weights go on the idle scalar queue so the
        # first window's loads aren't stuck behind 14MB of weight traffic.
        for kc in range(6):
            nc.sync.dma_start(out=wqkv[:, kc, :], in_=wqkv_d.ap()[kc])
        for kc in range(6):
            nc.scalar.dma_start(out=wproj[:, kc, :], in_=wproj_d.ap()[kc])
            nc.scalar.dma_start(out=w1[:, kc, :], in_=w1_d.ap()[kc])
        for kc in range(24):
            nc.scalar.dma_start(out=w2[:, kc, :], in_=w2_d.ap()[kc])
        bqkv = wp.tile([128, 18], F32)
        nc.sync.dma_start(out=bqkv, in_=bqkv_d.ap())
        relhw = wp.tile([128, 2, 27], BF)
        nc.sync.dma_start(out=relhw, in_=relhw_d.ap())
        bproj = wp.tile([1, DIM], BF)
        nc.sync.dma_start(out=bproj, in_=bproj_d.ap())
        b1c = wp.tile([128, 24], F32)
        nc.sync.dma_start(out=b1c, in_=b1_d.ap())
        b2r = wp.tile([1, DIM], BF)
        nc.sync.dma_start(out=b2r, in_=b2_d.ap())

        ident = wp.tile([128, 128], BF)
        make_identity(nc, ident)
        ones_col = wp.tile([128, 1], F32)
        nc.vector.memset(ones_col, 1.0)
        ones_1x128 = wp.tile([1, 128], BF)
        nc.vector.memset(ones_1x128, 1.0)
        ones_row = wp.tile([1, 512], BF)
        nc.vector.memset(ones_row, 1.0)
        eps_col = wp.tile([128, 1], F32)
        nc.vector.memset(eps_col, EPS)

        def ln_stats(src3, stat_chunks, mrrow):
            """src3: AP [128, 6, X]; writes mrrow [1, 2, TW] bf16 (mean, rstd)."""
            for (t0, csz) in stat_chunks:
                mps = ps_st.tile([128, 2], F32, tag="stat", name="mps")
                for dc in range(6):
                    nc.tensor.matmul(mps[:csz, 0:1], src3[:, dc, t0:t0 + csz],
                                     ones_col, start=(dc == 0), stop=(dc == 5))
                for dc in range(6):
                    sq = sb2.tile([128, 128], F32, tag="sq")
                    nc.vector.tensor_mul(out=sq[:, :csz],
                                         in0=src3[:, dc, t0:t0 + csz],
                                         in1=src3[:, dc, t0:t0 + csz])
                    nc.tensor.matmul(mps[:csz, 1:2], sq[:, :csz],
                                     ones_col, start=(dc == 0), stop=(dc == 5))
                mf = sb2.tile([128, 4], F32, tag="lncol")
                nc.scalar.mul(mf[:csz, 0:1], mps[:csz, 0:1], 1.0 / DIM)
                nc.scalar.mul(mf[:csz, 1:2], mps[:csz, 1:2], 1.0 / DIM)
                nc.vector.tensor_mul(out=mf[:csz, 2:3], in0=mf[:csz, 0:1],
                                     in1=mf[:csz, 0:1])
                nc.vector.tensor_sub(out=mf[:csz, 3:4], in0=mf[:csz, 1:2],
                                     in1=mf[:csz, 2:3])
                sd = sb2.tile([128, 1], F32, tag="lnsd")
                nc.scalar.activation(out=sd[:csz], in_=mf[:csz, 3:4],
                                     func=AF.Sqrt, bias=eps_col[:csz])
                rf = sb2.tile([128, 1], F32, tag="lnrf")
                nc.vector.reciprocal(out=rf[:csz], in_=sd[:csz])
                mb16 = sb2.tile([128, 2], BF, tag="lnb16")
                nc.vector.tensor_copy(out=mb16[:csz, 0:1], in_=mf[:csz, 0:1])
                nc.vector.tensor_copy(out=mb16[:csz, 1:2], in_=rf[:csz])
                nc.gpsimd.dma_start(out=mrrow[0:1, t0:t0 + csz, :],
                                    in_=mb16[:csz, :])

        def bcast_rows(mrrow, nfree):
            """-> sbuf f32 [128, 2, TW] with [:,0,:]=mean bcast, [:,1,:]=rstd."""
            bp = ps_bc.tile([128, 2, TW], F32, tag="bc")
            nc.tensor.matmul(bp[:, 0, :nfree], ones_1x128,
                             mrrow[0:1, :nfree, 0], start=True, stop=True)
            nc.tensor.matmul(bp[:, 1, :nfree], ones_1x128,
                             mrrow[0:1, :nfree, 1], start=True, stop=True)
            mbs = sb2.tile([128, 2, TW], F32, tag="mrbs")
            nc.vector.tensor_copy(out=mbs[:, :, :nfree], in_=bp[:, :, :nfree])
            return mbs

        def stage1(s):
            rh, rw = classes[s]
            nreal = rh * rw
            tok0 = s * TW
            chunks = attn_chunks(rh, rw)
            # ---- load xw^T ----
            xw = sb.tile([128, 6, TW], F32, tag="xw")
            for dc in range(6):
                nc.sync.dma_start(
                    out=xw[:, dc, :],
                    in_=xwT_d.ap()[128 * dc:128 * (dc + 1), tok0:tok0 + TW])

            # ---- LN1 (full 196 so pads normalize to exact 0) ----
            mr1 = sb2.tile([1, TW, 2], BF, tag="mr1")
            ln_stats(xw, [(0, 98), (98, 98)], mr1)
            mrb1 = bcast_rows(mr1, TW)
            xs = sb.tile([128, 6, TW], BF, tag="xs")
            for dc in range(6):
                tscr = sb2.tile([128, TW], F32, tag="tscr")
                nc.vector.tensor_sub(out=tscr, in0=xw[:, dc, :],
                                     in1=mrb1[:, 0, :])
                nc.vector.tensor_mul(out=xs[:, dc, :], in0=tscr,
                                     in1=mrb1[:, 1, :])

            # ---- qkv^T ----
            qkvT = sb.tile([128, 18, TW], BF, tag="qkvT")
            for mc in range(18):
                nfree = nreal if mc < 6 else TW
                qp = ps_mm.tile([128, 512], F32, tag="mm")
                for kc in range(6):
                    nc.tensor.matmul(qp[:, :nfree],
                                     wqkv[:, kc, 128 * mc:128 * (mc + 1)],
                                     xs[:, kc, :nfree],
                                     start=(kc == 0), stop=(kc == 5))
                nc.scalar.activation(out=qkvT[:, mc, :nfree],
                                     in_=qp[:, :nfree],
                                     func=AF.Identity,
                                     bias=bqkv[:, mc:mc + 1])

            # ---- rel-pos: EP[i, ri, h, d] = exp(q_i . relpos_rev) ----
            epd = dr.tile([TW, 2, NH, 27], BF, tag="epd", name="epd")
            for ci, (t0, csz) in enumerate(chunks):
                etk = sb.tile([128, 2, NH, 27], BF, tag=f"etk{ci}",
                              name=f"etk{ci}")
                for h in range(NH):
                    po = 64 * (h % 2)
                    pp = ps_mm.tile([128, 512], F32, tag="mm")
                    nc.tensor.matmul(
                        pp[:csz, :54],
                        qkvT[po:po + 64, h // 2, t0:t0 + csz],
                        relhw[po:po + 64, :, :],
                        start=True, stop=True)
                    nc.scalar.activation(
                        out=etk[:csz, :, h, :],
                        in_=pp[:csz, :54].rearrange("p (r d) -> p r d", r=2),
                        func=AF.Exp)
                nc.gpsimd.dma_start(out=epd[t0:t0 + csz, :, :, :],
                                    in_=etk[:csz, :, :, :])

            # ---- gather EC (per-row Toeplitz DMAs) ----
            PITCH = 2 * NH * 27  # 648
            ec = {}
            for ci, (t0, csz) in enumerate(chunks):
                nrows = csz // rw
                r_base = t0 // rw
                base = epd[:, :, :, :]
                for ri in (0, 1):
                    ect = sb.tile([128, NH, 14], BF, tag=f"ec{ri}{ci}",
                                  name=f"ec{ri}{ci}")
                    ec[(ri, ci)] = ect
                    for j in range(nrows):
                        r = r_base + j
                        if ri == 0:
                            off = (r * rw) * PITCH + 13 - r
                            stride0 = PITCH
                        else:
                            off = (r * rw) * PITCH + 324 + 13
                            stride0 = PITCH - 1
                        in_ap = bass.AP(
                            tensor=base.tensor,
                            offset=base.offset + off,
                            ap=[[stride0, rw], [27, NH], [1, 14]])
                        nc.gpsimd.dma_start(
                            out=ect[j * rw:(j + 1) * rw, :, :], in_=in_ap)


            return dict(xw=xw, qkvT=qkvT, ec=ec, chunks=chunks, rh=rh,
                        rw=rw, nreal=nreal, tok0=tok0)

        def stage2(s, t, ytile, ynb, yoff):
            rh = t["rh"]; rw = t["rw"]; nreal = t["nreal"]
            chunks = t["chunks"]; xw = t["xw"]; qkvT = t["qkvT"]; ec = t["ec"]
            # ---- attention ----
            avT = sb1.tile([128, 6, TW], BF, tag="avT")
            for hp in range(6):
                vt = []
                for (j0, jn) in ((0, 128), (128, 68)):
                    pv = ps_tr.tile([128, 128], BF, tag="tr")
                    nc.tensor.transpose(pv[:jn, :],
                                        qkvT[:, 12 + hp, j0:j0 + jn],
                                        ident)
                    vtk = sb.tile([128, 128], BF, tag=f"vtk{j0}")
                    nc.vector.tensor_copy(out=vtk[:jn, :], in_=pv[:jn, :])
                    vt.append((j0, jn, vtk))
                avp = ps_av.tile([128, TW], F32, tag="av")
                for hh in (0, 1):
                    h = 2 * hp + hh
                    ET = [sb.tile([128, TW], BF, tag="ET0", name="ET0"),
                          sb.tile([128, TW], BF, tag="ET1", name="ET1")]
                    for ci, (t0, csz) in enumerate(chunks):
                        sp = ps_mm.tile([128, 512], F32, tag="mm")
                        nc.tensor.matmul(
                            sp[:csz, :TW],
                            qkvT[64 * hh:64 * hh + 64, h // 2, t0:t0 + csz],
                            qkvT[64 * hh:64 * hh + 64, 6 + h // 2, :],
                            start=True, stop=True)
                        E = sb.tile([128, TW], BF, tag=f"E{ci}")
                        nc.scalar.activation(out=E[:csz, :], in_=sp[:csz, :TW],
                                             func=AF.Exp)
                        for (cst, nr2, r20, nc2, c20) in col_groups(rh, rw):
                            ev = E[:csz, cst:cst + nr2 * nc2].rearrange(
                                "p (a b) -> p a b", a=nr2)
                            nc.vector.tensor_mul(
                                out=ev, in0=ev,
                                in1=ec[(0, ci)][:csz, h, r20:r20 + nr2]
                                [:, :, None].broadcast_to([csz, nr2, nc2]))
                            nc.vector.tensor_mul(
                                out=ev, in0=ev,
                                in1=ec[(1, ci)][:csz, h, c20:c20 + nc2]
                                [:, None, :].broadcast_to([csz, nr2, nc2]))
                        zt = sb2.tile([128, 2], F32, tag="z")
                        nc.vector.reduce_sum(out=zt[:csz, 0:1],
                                             in_=E[:csz, :], axis=AX.X)
                        nc.vector.reciprocal(out=zt[:csz, 1:2],
                                             in_=zt[:csz, 0:1])
                        nc.vector.tensor_scalar_mul(out=E[:csz, :],
                                                    in0=E[:csz, :],
                                                    scalar1=zt[:csz, 1:2])
                        for ji, (j0, jn) in enumerate(((0, 128), (128, 68))):
                            pe = ps_tr.tile([128, 128], BF, tag="tr")
                            nc.tensor.transpose(pe[:jn, :csz],
                                                E[:csz, j0:j0 + jn],
                                                ident[:csz, :csz])
                            nc.vector.tensor_copy(
                                out=ET[ji][:jn, t0:t0 + csz],
                                in_=pe[:jn, :csz])
                    for ji, (j0, jn, vtk) in enumerate(vt):
                        nc.tensor.matmul(
                            avp[64 * hh:64 * hh + 64, :nreal],
                            vtk[:jn, 64 * hh:64 * hh + 64],
                            ET[ji][:jn, :nreal],
                            start=(ji == 0), stop=(ji == 1))
                nc.vector.tensor_copy(out=avT[:, hp, :nreal],
                                      in_=avp[:, :nreal])

            # ---- proj + residual -> y ----
            for oc in range(6):
                zp = ps_mm.tile([128, 512], F32, tag="mm")
                nc.tensor.matmul(zp[:, :nreal],
                                 bproj[0:1, 128 * oc:128 * (oc + 1)],
                                 ones_row[0:1, :nreal],
                                 start=True, stop=False)
                for kc in range(6):
                    nc.tensor.matmul(zp[:, :nreal],
                                     wproj[:, kc, 128 * oc:128 * (oc + 1)],
                                     avT[:, kc, :nreal],
                                     start=False, stop=(kc == 5))
                nc.vector.tensor_add(out=ytile[:, oc, yoff:yoff + nreal],
                                     in0=xw[:, oc, :nreal],
                                     in1=zp[:, :nreal])

            # ---- LN2 (real cols) -> ynb ----
            mr2 = sb2.tile([1, TW, 2], BF, tag="mr2")
            ysub = ytile[:, :, yoff:yoff + nreal]
            ln_stats(ysub, chunks, mr2)
            mrb2 = bcast_rows(mr2, nreal)
            for dc in range(6):
                tscr = sb2.tile([128, TW], F32, tag="tscr")
                nc.vector.tensor_sub(out=tscr[:, :nreal],
                                     in0=ysub[:, dc, :],
                                     in1=mrb2[:, 0, :nreal])
                nc.vector.tensor_mul(out=ynb[:, dc, yoff:yoff + nreal],
                                     in0=tscr[:, :nreal],
                                     in1=mrb2[:, 1, :nreal])


        tinfo = {}
        tinfo[0] = stage1(0)
        if n_windows > 1:
            tinfo[1] = stage1(1)
        next_s1 = 2
        for grp in groups:
            ytile = sb1.tile([128, 6, 392], F32, tag="y", name="y")
            ynb = sb1.tile([128, 6, 392], BF, tag="ynb", name="ynb")
            offs = []
            o = 0
            for s_ in grp:
                offs.append(o)
                o += classes[s_][0] * classes[s_][1]
            np_grp = o
            for wi, s_ in enumerate(grp):
                stage2(s_, tinfo.pop(s_), ytile, ynb, offs[wi])
                if next_s1 < n_windows:
                    tinfo[next_s1] = stage1(next_s1)
                    next_s1 += 1
            # ---- MLP on packed group ----
            hT = sb1.tile([128, 24, 392], BF, tag="hT")
            for mc in range(24):
                p1 = ps_mm.tile([128, 512], F32, tag="mm")
                for kc in range(6):
                    nc.tensor.matmul(p1[:, :np_grp],
                                     w1[:, kc, 128 * mc:128 * (mc + 1)],
                                     ynb[:, kc, :np_grp],
                                     start=(kc == 0), stop=(kc == 5))
                if exact_gelu:
                    nc.scalar.activation(out=hT[:, mc, :np_grp],
                                         in_=p1[:, :np_grp],
                                         func=AF.Gelu, bias=b1c[:, mc:mc + 1])
                else:
                    # sim-only tanh-approx gelu composed from sim-supported ops
                    u = sb2.tile([128, 392], F32, tag="gu", bufs=1)
                    nc.scalar.activation(out=u[:, :np_grp], in_=p1[:, :np_grp],
                                         func=AF.Identity,
                                         bias=b1c[:, mc:mc + 1])
                    a = sb2.tile([128, 392], F32, tag="ga", bufs=1)
                    nc.vector.tensor_mul(out=a[:, :np_grp], in0=u[:, :np_grp],
                                         in1=u[:, :np_grp])
                    nc.vector.tensor_mul(out=a[:, :np_grp], in0=a[:, :np_grp],
                                         in1=u[:, :np_grp])
                    arg = sb2.tile([128, 392], F32, tag="garg", bufs=1)
                    nc.scalar.mul(arg[:, :np_grp], a[:, :np_grp], 0.044715)
                    nc.vector.tensor_add(out=arg[:, :np_grp], in0=arg[:, :np_grp],
                                         in1=u[:, :np_grp])
                    nc.scalar.activation(out=arg[:, :np_grp], in_=arg[:, :np_grp],
                                         func=AF.Tanh, scale=0.7978845608)
                    nc.scalar.activation(out=arg[:, :np_grp], in_=arg[:, :np_grp],
                                         func=AF.Identity, bias=1.0, scale=1.0)
                    nc.scalar.mul(arg[:, :np_grp], arg[:, :np_grp], 0.5)
                    nc.vector.tensor_mul(out=hT[:, mc, :np_grp],
                                         in0=u[:, :np_grp], in1=arg[:, :np_grp])
            for oc in range(6):
                p2 = ps_mm.tile([128, 512], F32, tag="mm")
                nc.tensor.matmul(p2[:, :np_grp],
                                 b2r[0:1, 128 * oc:128 * (oc + 1)],
                                 ones_row[0:1, :np_grp], start=True, stop=False)
                for kc in range(24):
                    nc.tensor.matmul(p2[:, :np_grp],
                                     w2[:, kc, 128 * oc:128 * (oc + 1)],
                                     hT[:, kc, :np_grp],
                                     start=False, stop=(kc == 23))
                fo = sb2.tile([128, 392], F32, tag="fo")
                nc.vector.tensor_add(out=fo[:, :np_grp],
                                     in0=ytile[:, oc, :np_grp],
                                     in1=p2[:, :np_grp])
                for wi, s in enumerate(grp):
                    rh, rw = classes[s]
                    nc.sync.dma_start(
                        out=out_d.ap()[128 * oc:128 * (oc + 1),
                                       s * TW:s * TW + rh * rw],
                        in_=fo[:, offs[wi]:offs[wi] + rh * rw])

    nc.compile()
    return nc


# ----------------------------------------------------------------------------
# host wrapper
# ----------------------------------------------------------------------------

def _window_assignment():
    interior = [(b, wy, wx) for b in range(B) for wy in range(4) for wx in range(4)]
    right = [(b, wy, 4) for b in range(B) for wy in range(4)]
    bottom = [(b, 4, wx) for b in range(B) for wx in range(4)]
    corner = [(b, 4, 4) for b in range(B)]
    cores = []
    for c in range(NCORES):
        wins = interior[8 * c:8 * c + 8] + right[2 * c:2 * c + 2] \
            + bottom[2 * c:2 * c + 2]
        wins.append(corner[c] if c < 4 else None)
        cores.append(wins)
    return cores


def _prep_consts(ln1_w, ln1_b, qkv_w, qkv_b, proj_w, proj_b, rel_pos_h,
                 rel_pos_w, ln2_w, ln2_b, w1, b1, w2, b2):
    qkv_w = np.asarray(qkv_w, np.float32)
    w1 = np.asarray(w1, np.float32)
    Wq = np.asarray(ln1_w, np.float32)[:, None] * qkv_w
    bq = np.asarray(qkv_b, np.float32) + np.asarray(ln1_b, np.float32) @ qkv_w
    Wq = Wq.copy()
    bq = bq.copy()
    Wq[:, :DIM] *= SCALE
    bq[:DIM] *= SCALE
    W1 = np.asarray(ln2_w, np.float32)[:, None] * w1
    B1 = np.asarray(b1, np.float32) + np.asarray(ln2_b, np.float32) @ w1
    return {
        "wqkv": Wq.reshape(6, 128, 3 * DIM).astype(BF16),
        "bqkv": np.ascontiguousarray(bq.reshape(18, 128).T).astype(np.float32),
        "relhw": np.ascontiguousarray(np.stack(
            [np.concatenate([np.asarray(t, np.float32)[::-1].T] * 2, axis=0)
             for t in (rel_pos_h, rel_pos_w)], axis=1)).astype(BF16),
        "wproj": np.asarray(proj_w, np.float32).reshape(6, 128, DIM).astype(BF16),
        "bproj": np.asarray(proj_b, np.float32).reshape(1, DIM).astype(BF16),
        "w1": W1.reshape(6, 128, MLP_H).astype(BF16),
        "b1": np.ascontiguousarray(B1.reshape(24, 128).T).astype(np.float32),
        "w2": np.asarray(w2, np.float32).reshape(24, 128, DIM).astype(BF16),
        "b2": np.asarray(b2, np.float32).reshape(1, DIM).astype(BF16),
    }


_ORDER_CACHE = {}


def _order_idx(rh, rw):
    key = (rh, rw)
    if key not in _ORDER_CACHE:
        _ORDER_CACHE[key] = np.array(token_order(rh, rw), np.int64)
    return _ORDER_CACHE[key]


def kernel(x, ln1_w, ln1_b, qkv_w, qkv_b, proj_w, proj_b, rel_pos_h,
           rel_pos_w, ln2_w, ln2_b, w1, b1, w2, b2):
    from concourse.bass_utils import run_bass_kernel_spmd

    x = np.asarray(x, np.float32)
    consts = _prep_consts(ln1_w, ln1_b, qkv_w, qkv_b, proj_w, proj_b,
                          rel_pos_h, rel_pos_w, ln2_w, ln2_b, w1, b1, w2, b2)

    if "nc" not in _CACHE:
        _CACHE["nc"] = _build()
    nc = _CACHE["nc"]

    assign = _window_assignment()
    xpad = np.zeros((B, 70, 70, DIM), np.float32)
    xpad[:, :H, :W, :] = x

    in_maps = []
    for c in range(NCORES):
        xwT = np.zeros((DIM, CORE_TOK), np.float32)
        for s, win in enumerate(assign[c]):
            if win is None:
                continue
            b, wy, wx = win
            rh, rw = WCLASSES[s]
            idx = _order_idx(rh, rw)
            blk = xpad[b, 14 * wy:14 * wy + 14, 14 * wx:14 * wx + 14, :]
            xwT[:, s * TW:(s + 1) * TW] = blk[idx[:, 0], idx[:, 1], :].T
        m = {"xwT": xwT}
        m.update(consts)
        in_maps.append(m)

    res = run_bass_kernel_spmd(nc, in_maps, core_ids=list(range(NCORES)),
                               **_CACHE.get("run_kwargs", {}))
    _CACHE["last_result"] = res

    out = np.zeros((B, H, W, DIM), np.float32)
    for c in range(NCORES):
        oT = res.results[c]["outT"]
        for s, win in enumerate(assign[c]):
            if win is None:
                continue
            b, wy, wx = win
            rh, rw = WCLASSES[s]
            idx = _order_idx(rh, rw)[:rh * rw]
            out[b, 14 * wy + idx[:, 0], 14 * wx + idx[:, 1], :] = \
                oT[:, s * TW:s * TW + rh * rw].T
    return out



# revision 12
# speedup vs baseline: 1.2264x; 1.2264x over previous
"""Windowed-attention ViT block (SAM-style) on 8 TRN2 NeuronCores.

Feature-major ("^T") layout [dim, tokens] on device. Per core: 13 window
slots processed as 6 same-class PAIRS + 1 solo, so qkv/LN/proj/MLP run at
392-wide free dims. LN stats computed as ones-stationary row matmuls
(avoids fp32 wide-LDW stats matmuls); rstd via Ln+Exp (stays in the
exp table set). Rel-pos handled multiplicatively: exp(S+B) =
exp(S)*EC1*EC2 with EC gathered from exp(q @ relpos_rev) via padded DRAM
copies that make the Toeplitz gather a single 3-dim strided DMA per
(ri, chunk). QK/EP matmuls are issued as head pairs on disjoint 64-row
PE groups; AV is issued col-paired on 64-col groups.
"""
import numpy as np
import ml_dtypes
from contextlib import ExitStack

DIM = 768
NH = 12
HD = 64
WS = 14
H = W = 64
MLP_H = 3072
EPS = 1e-5
SCALE = HD ** -0.5
B = 4
NCORES = 8
TW = WS * WS  # 196
NW_CORE = 13
CORE_TOK = NW_CORE * TW  # 2548

WCLASSES = [(14, 14)] * 8 + [(14, 8)] * 2 + [(8, 14)] * 2 + [(8, 8)]
PAIRS = [(0, 1), (2, 3), (4, 5), (6, 7), (8, 9), (10, 11), (12,)]
REC = 2 * NH * 27  # 648 full token record; per-ri record = 324
HREC = NH * 27     # 324

BF16 = ml_dtypes.bfloat16


def token_order(rh, rw):
    order = [(r, c) for r in range(rh) for c in range(rw)]
    order += [(r, c) for r in range(rh) for c in range(rw, WS)]
    order += [(r, c) for r in range(rh, WS) for c in range(WS)]
    return order


def attn_chunks(rh, rw):
    """Token-offset chunks of the real block: (t0, csz); row-aligned."""
    if rh * rw == 196:
        return [(0, 98), (98, 98)]
    return [(0, rh * rw)]


def col_groups(rh, rw):
    """Key-column groups: (start, n_r2, r2_0, n_c2, c2_0)."""
    g = [(0, rh, 0, rw, 0)]
    if rw < WS:
        g.append((rh * rw, rh, 0, WS - rw, rw))
    if rh < WS:
        g.append((rh * WS, WS - rh, rh, WS, 0))
    return g


_CACHE = {}


def _build():
    import concourse.bass as bass
    import concourse.mybir as mybir
    import concourse.tile as tile
    from concourse import bacc
    from concourse.masks import make_identity

    F32 = mybir.dt.float32
    BF = mybir.dt.bfloat16
    AF = mybir.ActivationFunctionType
    AX = mybir.AxisListType

    classes = WCLASSES
    ncols = NW_CORE * TW

    nc = bacc.Bacc("TRN2", target_bir_lowering=False, debug=False,
                   enable_asserts=False, num_devices=NCORES)

    xwT_d = nc.dram_tensor("xwT", [DIM, ncols], BF, kind="ExternalInput")
    wqkv_d = nc.dram_tensor("wqkv", [6, 128, 3 * DIM], BF, kind="ExternalInput")
    bqkv_d = nc.dram_tensor("bqkv", [128, 18], F32, kind="ExternalInput")
    relhw_d = nc.dram_tensor("relhw", [128, 2, 27], BF, kind="ExternalInput")
    wproj_d = nc.dram_tensor("wproj", [6, 128, DIM], BF, kind="ExternalInput")
    bproj_d = nc.dram_tensor("bproj", [1, DIM], BF, kind="ExternalInput")
    w1_d = nc.dram_tensor("w1", [6, 128, MLP_H], BF, kind="ExternalInput")
    b1_d = nc.dram_tensor("b1", [128, 24], F32, kind="ExternalInput")
    w2_d = nc.dram_tensor("w2", [24, 128, DIM], BF, kind="ExternalInput")
    b2_d = nc.dram_tensor("b2", [1, DIM], BF, kind="ExternalInput")
    out_d = nc.dram_tensor("outT", [DIM, ncols], F32, kind="ExternalOutput")

    with tile.TileContext(nc) as tc, ExitStack() as ctx:
        wp = ctx.enter_context(tc.tile_pool(name="weights", bufs=1))
        sb = ctx.enter_context(tc.tile_pool(name="sb", bufs=2))
        sb1 = ctx.enter_context(tc.tile_pool(name="sb1", bufs=1))
        sb2 = ctx.enter_context(tc.tile_pool(name="sb2", bufs=2))
        ps_mm = ctx.enter_context(tc.tile_pool(name="ps_mm", bufs=3, space="PSUM"))
        ps_at = ctx.enter_context(tc.tile_pool(name="ps_at", bufs=2, space="PSUM"))
        ps_av = ctx.enter_context(tc.tile_pool(name="ps_av", bufs=1, space="PSUM"))
        ps_rw = ctx.enter_context(tc.tile_pool(name="ps_rw", bufs=1, space="PSUM"))
        dr = ctx.enter_context(tc.tile_pool(name="dr", bufs=2, space="DRAM"))

        # ---- constants ----
        wqkv = wp.tile([128, 6, 3 * DIM], BF)
        wproj = wp.tile([128, 6, DIM], BF)
        w1 = wp.tile([128, 6, MLP_H], BF)
        w2 = wp.tile([128, 24, DIM], BF)
        # wqkv on the sync queue (needed first, ahead of window-0 xw);
        # bulky later-phase weights go on the idle scalar queue.
        for kc in range(6):
            nc.sync.dma_start(out=wqkv[:, kc, :], in_=wqkv_d.ap()[kc])
        for kc in range(6):
            nc.scalar.dma_start(out=wproj[:, kc, :], in_=wproj_d.ap()[kc])
            nc.scalar.dma_start(out=w1[:, kc, :], in_=w1_d.ap()[kc])
        for kc in range(24):
            nc.scalar.dma_start(out=w2[:, kc, :], in_=w2_d.ap()[kc])
        bqkv = wp.tile([128, 18], F32)
        nc.sync.dma_start(out=bqkv, in_=bqkv_d.ap())
        relhw = wp.tile([128, 2, 27], BF)
        nc.sync.dma_start(out=relhw, in_=relhw_d.ap())
        bproj = wp.tile([1, DIM], BF)
        nc.sync.dma_start(out=bproj, in_=bproj_d.ap())
        b1c = wp.tile([128, 24], F32)
        nc.sync.dma_start(out=b1c, in_=b1_d.ap())
        b2r = wp.tile([1, DIM], BF)
        nc.sync.dma_start(out=b2r, in_=b2_d.ap())

        ident = wp.tile([128, 128], BF)
        make_identity(nc, ident)
        ones_col = wp.tile([128, 1], F32)
        nc.vector.memset(ones_col, 1.0)
        ones_colb = wp.tile([128, 1], BF)
        nc.vector.memset(ones_colb, 1.0)
        ones_1x128 = wp.tile([1, 128], BF)
        nc.vector.memset(ones_1x128, 1.0)
        ones_row = wp.tile([1, 512], BF)
        nc.vector.memset(ones_row, 1.0)
        eps_c = wp.tile([1, 1], F32)
        nc.vector.memset(eps_c, EPS)

        # warm the PE HAM while initial weight DMAs are in flight
        wu = ps_mm.tile([128, 512], F32, tag="mm", name="warm")
        for _ in range(48):
            nc.tensor.matmul(wu[:, :128], ident, ident, start=True, stop=True)

        def ln_rows(src3, nfree, f32src):
            """LN stats over partition dim via ones-stationary matmuls.

            src3: AP [128, 6, X]. Returns sbuf rows tile [1, 2, nfree]
            bf16 rows (mean, rstd)."""
            sum_ps = ps_rw.tile([1, 512], F32, tag="lnsum", name="lnsum")
            sq_ps = ps_rw.tile([1, 512], F32, tag="lnsq", name="lnsq")
            lnc = ones_col if f32src else ones_colb
            for dc in range(6):
                nc.tensor.matmul(sum_ps[0:1, :nfree], lnc,
                                 src3[:, dc, :nfree],
                                 start=(dc == 0), stop=(dc == 5))
            for dc in range(6):
                sq = sb2.tile([128, 392], BF, tag="sq", bufs=1)
                nc.vector.tensor_mul(out=sq[:, :nfree],
                                     in0=src3[:, dc, :nfree],
                                     in1=src3[:, dc, :nfree])
                nc.tensor.matmul(sq_ps[0:1, :nfree], ones_colb,
                                 sq[:, :nfree],
                                 start=(dc == 0), stop=(dc == 5))
            rows = sb2.tile([1, 2, 392], BF, tag="lnrows")
            # mean = sum/768
            nc.scalar.mul(rows[0:1, 0, :nfree], sum_ps[0:1, :nfree], 1.0 / DIM)
            # var = sq/768 - mean^2 ; rstd = exp(-0.5*ln(var+eps))
            vr = sb2.tile([1, 2, 392], F32, tag="lnvr", bufs=1)
            nc.vector.tensor_mul(out=vr[0:1, 0, :nfree],
                                 in0=rows[0:1, 0, :nfree],
                                 in1=rows[0:1, 0, :nfree])
            nc.scalar.mul(vr[0:1, 1, :nfree], sq_ps[0:1, :nfree], 1.0 / DIM)
            nc.vector.tensor_sub(out=vr[0:1, 1, :nfree],
                                 in0=vr[0:1, 1, :nfree],
                                 in1=vr[0:1, 0, :nfree])
            nc.scalar.activation(out=vr[0:1, 0, :nfree],
                                 in_=vr[0:1, 1, :nfree],
                                 func=AF.Ln, bias=eps_c[0:1])
            nc.scalar.activation(out=rows[0:1, 1, :nfree],
                                 in_=vr[0:1, 0, :nfree],
                                 func=AF.Exp, scale=-0.5)
            return rows

        def ln_apply(src3, dst3, rows, nfree):
            """dst = (src - mean) * rstd, with mean/rstd bcast via matmul."""
            bpm = ps_mm.tile([128, 512], F32, tag="mm", name="lnbm")
            bpr = ps_mm.tile([128, 512], F32, tag="mm", name="lnbr")
            nc.tensor.matmul(bpm[:, :nfree], ones_1x128,
                             rows[0:1, 0, :nfree], start=True, stop=True)
            nc.tensor.matmul(bpr[:, :nfree], ones_1x128,
                             rows[0:1, 1, :nfree], start=True, stop=True)
            for dc in range(6):
                tscr = sb2.tile([128, 392], F32, tag="tscr", bufs=1)
                nc.vector.tensor_sub(out=tscr[:, :nfree],
                                     in0=src3[:, dc, :nfree],
                                     in1=bpm[:, :nfree])
                nc.vector.tensor_mul(out=dst3[:, dc, :nfree],
                                     in0=tscr[:, :nfree],
                                     in1=bpr[:, :nfree])

        def stage1(pi):
            """Load + LN1 + qkv for a pair; EP/epd/gather per window."""
            grp = PAIRS[pi]
            nw = len(grp)
            nfree = nw * TW
            tok0 = grp[0] * TW
            rh, rw = classes[grp[0]]
            nreal = rh * rw
            chunks = attn_chunks(rh, rw)

            # ---- load xw^T (pair-wide) ----
            xw = sb.tile([128, 6, 392], BF, tag="xw")
            for dc in range(6):
                nc.sync.dma_start(
                    out=xw[:, dc, :nfree],
                    in_=xwT_d.ap()[128 * dc:128 * (dc + 1), tok0:tok0 + nfree])

            # ---- LN1 (full cols so pads normalize to exact 0) ----
            rows1 = ln_rows(xw, nfree, False)
            xs = sb.tile([128, 6, 392], BF, tag="xs", bufs=1)
            ln_apply(xw, xs, rows1, nfree)

            # ---- qkv^T (pair-wide) ----
            qkvT = sb.tile([128, 18, 392], BF, tag="qkvT")
            for mc in range(18):
                qp = ps_mm.tile([128, 512], F32, tag="mm")
                for kc in range(6):
                    nc.tensor.matmul(qp[:, :nfree],
                                     wqkv[:, kc, 128 * mc:128 * (mc + 1)],
                                     xs[:, kc, :nfree],
                                     start=(kc == 0), stop=(kc == 5))
                nc.scalar.activation(out=qkvT[:, mc, :nfree],
                                     in_=qp[:, :nfree],
                                     func=AF.Identity,
                                     bias=bqkv[:, mc:mc + 1])

            # ---- rel-pos EP + padded epd + gather, per window ----
            ec = {}
            for wi, s in enumerate(grp):
                woff = wi * TW
                # EP[q, ri, h, s27] = exp(q . relpos_rev), head-pair packed
                etks = []
                for ci, (t0, csz) in enumerate(chunks):
                    etk = sb.tile([128, 2, NH, 27], BF, tag=f"etk{wi}{ci}",
                                  name=f"etk{wi}{ci}", bufs=1)
                    etks.append(etk)
                    for hp in range(6):
                        pps = []
                        for par in range(2):
                            po = 64 * par
                            pp = ps_mm.tile([128, 512], F32, tag="mm")
                            nc.tensor.matmul(
                                pp[:csz, :54],
                                qkvT[po:po + 64, hp, woff + t0:woff + t0 + csz],
                                relhw[po:po + 64, :, :],
                                start=True, stop=True)
                            pps.append(pp)
                        for par in range(2):
                            nc.scalar.activation(
                                out=etk[:csz, :, 2 * hp + par, :],
                                in_=pps[par][:csz, :54].rearrange(
                                    "p (r d) -> p r d", r=2),
                                func=AF.Exp)
                # Padded DRAM copies make the Toeplitz gather a uniform
                # strided read: ri0 places token records at tok*324 + row
                # (row stride rw*324+1), so record + (13 - row + kh) is
                # linear at stride 324. ri1 places records at tok*325 + j
                # (j = in-row pos; within-row stride 326, row stride
                # rw*325), so record + (13 - j + kw) is linear at 325.
                nrows_all = (TW if rh * rw == 196 else nreal) // rw
                ntok = nrows_all * rw
                sz0 = nrows_all * (rw * HREC + 1) + 350
                sz1 = ntok * 325 + 350
                epd0 = dr.tile([sz0], BF, tag=f"epd0_{wi}",
                               name=f"epd0_{wi}")
                epd1 = dr.tile([sz1], BF, tag=f"epd1_{wi}",
                               name=f"epd1_{wi}")
                b0 = epd0[:]
                b1 = epd1[:]
                for ci, (t0, csz) in enumerate(chunks):
                    nr = csz // rw
                    r0 = t0 // rw
                    dst0 = bass.AP(
                        tensor=b0.tensor,
                        offset=b0.offset + r0 * (rw * HREC + 1),
                        ap=[[rw * HREC + 1, nr], [HREC, rw], [1, HREC]])
                    nc.gpsimd.dma_start(out=dst0,
                                        in_=etks[ci][:csz, 0, :, :])
                    dst1 = bass.AP(
                        tensor=b1.tensor,
                        offset=b1.offset + t0 * 325,
                        ap=[[rw * 325, nr], [326, rw], [1, HREC]])
                    nc.gpsimd.dma_start(out=dst1,
                                        in_=etks[ci][:csz, 1, :, :])
                for ci, (t0, csz) in enumerate(chunks):
                    e0 = sb.tile([128, NH, 14], BF, tag=f"ec0{ci}{wi}",
                                 name=f"ec0{ci}{wi}")
                    src0 = bass.AP(
                        tensor=b0.tensor, offset=b0.offset + t0 * HREC + 13,
                        ap=[[HREC, csz], [27, NH], [1, 14]])
                    nc.sync.dma_start(out=e0[:csz, :, :], in_=src0)
                    e1 = sb.tile([128, NH, 14], BF, tag=f"ec1{ci}{wi}",
                                 name=f"ec1{ci}{wi}")
                    src1 = bass.AP(
                        tensor=b1.tensor, offset=b1.offset + t0 * 325 + 13,
                        ap=[[325, csz], [27, NH], [1, 14]])
                    nc.scalar.dma_start(out=e1[:csz, :, :], in_=src1)
                    ec[(0, ci, wi)] = e0
                    ec[(1, ci, wi)] = e1

            return dict(xw=xw, qkvT=qkvT, ec=ec, chunks=chunks, rh=rh,
                        rw=rw, nreal=nreal, grp=grp, nfree=nfree)

        def stage2(t, wi, avT):
            """Attention for window wi of pair t -> avT[:, :, wi*TW...]."""
            rh = t["rh"]; rw = t["rw"]; nreal = t["nreal"]
            chunks = t["chunks"]; qkvT = t["qkvT"]; ec = t["ec"]
            woff = wi * TW
            cg = col_groups(rh, rw)
            for hp in range(6):
                # v^T tiles (both heads of the pair)
                vt = []
                for (j0, jn) in ((0, 128), (128, 68)):
                    pv = ps_at.tile([128, 2, 196], BF, tag="at", name="vtr")
                    nc.tensor.transpose(pv[:jn, 0, :128],
                                        qkvT[:, 12 + hp, woff + j0:woff + j0 + jn],
                                        ident)
                    vtk = sb.tile([128, 128], BF, tag=f"vtk{j0}")
                    nc.vector.tensor_copy(out=vtk[:jn, :], in_=pv[:jn, 0, :128])
                    vt.append((j0, jn, vtk))
                # scores for BOTH heads first: row-group pairs run
                # concurrently on the PE (lhsT base partition 0 vs 64)
                sps = []
                for hh in (0, 1):
                    po = 64 * hh
                    sp = ps_at.tile([128, 2, 196], F32, tag="at",
                                    name=f"sp{hh}")
                    for ci, (t0, csz) in enumerate(chunks):
                        nc.tensor.matmul(
                            sp[:csz, ci, :],
                            qkvT[po:po + 64, hp, woff + t0:woff + t0 + csz],
                            qkvT[po:po + 64, 6 + hp, woff:woff + TW],
                            start=True, stop=True)
                    sps.append(sp)
                ET = {}
                for hh in (0, 1):
                    h = 2 * hp + hh
                    sp = sps[hh]
                    E = sb.tile([128, 2, 196], BF, tag="E", name="E")
                    zt = sb2.tile([128, 4], F32, tag="z")
                    for ci, (t0, csz) in enumerate(chunks):
                        nc.scalar.activation(out=E[:csz, ci, :],
                                             in_=sp[:csz, ci, :],
                                             func=AF.Exp)
                        for (cst, nr2, r20, nc2, c20) in cg:
                            ev = E[:csz, ci, cst:cst + nr2 * nc2].rearrange(
                                "p (a b) -> p a b", a=nr2)
                            nc.vector.tensor_mul(
                                out=ev, in0=ev,
                                in1=ec[(0, ci, wi)][:csz, h, r20:r20 + nr2]
                                [:, :, None].broadcast_to([csz, nr2, nc2]))
                            nc.vector.tensor_mul(
                                out=ev, in0=ev,
                                in1=ec[(1, ci, wi)][:csz, h, c20:c20 + nc2]
                                [:, None, :].broadcast_to([csz, nr2, nc2]))
                        nc.vector.reduce_sum(out=zt[:csz, ci:ci + 1],
                                             in_=E[:csz, ci, :], axis=AX.X)
                    nch = len(chunks)
                    csz0 = chunks[0][1]
                    nc.vector.reciprocal(out=zt[:csz0, 2:2 + nch],
                                         in_=zt[:csz0, 0:nch])
                    ETt = [sb.tile([128, 196], BF, tag="ET0", name="ET0"),
                           sb.tile([128, 196], BF, tag="ET1", name="ET1")]
                    ET[hh] = ETt
                    for ci, (t0, csz) in enumerate(chunks):
                        nc.vector.tensor_scalar_mul(
                            out=E[:csz, ci, :], in0=E[:csz, ci, :],
                            scalar1=zt[:csz, 2 + ci:3 + ci])
                        for ji, (j0, jn) in enumerate(((0, 128), (128, 68))):
                            pe = ps_at.tile([128, 2, 196], BF, tag="at",
                                            name="pe")
                            nc.tensor.transpose(pe[:jn, 0, :csz],
                                                E[:csz, ci, j0:j0 + jn],
                                                ident[:csz, :csz])
                            nc.vector.tensor_copy(
                                out=ETt[ji][:jn, t0:t0 + csz],
                                in_=pe[:jn, 0, :csz])
                # AV, col-paired across hh
                avp = ps_av.tile([128, 196], F32, tag="av")
                for ji, (j0, jn, vtk) in enumerate(vt):
                    for hh in (0, 1):
                        nc.tensor.matmul(
                            avp[64 * hh:64 * hh + 64, :nreal],
                            vtk[:jn, 64 * hh:64 * hh + 64],
                            ET[hh][ji][:jn, :nreal],
                            start=(ji == 0), stop=(ji == 1),
                            tile_position=(0, 64 * hh))
                nc.vector.tensor_copy(out=avT[:, hp, woff:woff + nreal],
                                      in_=avp[:, :nreal])

        def proj_mlp(t, avT, offs, np_grp):
            """proj + residual + LN2 + MLP for the pair, packed cols."""
            grp = t["grp"]; xw = t["xw"]; nreal = t["nreal"]
            nw = len(grp)
            span = (nw - 1) * TW + nreal  # proj free span incl. slot gap
            ytile = sb1.tile([128, 6, 392], F32, tag="y", name="y")
            for oc in range(6):
                zp = ps_mm.tile([128, 512], F32, tag="mm")
                nc.tensor.matmul(zp[:, :span],
                                 bproj[0:1, 128 * oc:128 * (oc + 1)],
                                 ones_row[0:1, :span],
                                 start=True, stop=False)
                for kc in range(6):
                    nc.tensor.matmul(zp[:, :span],
                                     wproj[:, kc, 128 * oc:128 * (oc + 1)],
                                     avT[:, kc, :span],
                                     start=False, stop=(kc == 5))
                for wi in range(nw):
                    nc.vector.tensor_add(
                        out=ytile[:, oc, offs[wi]:offs[wi] + nreal],
                        in0=xw[:, oc, wi * TW:wi * TW + nreal],
                        in1=zp[:, wi * TW:wi * TW + nreal])

            # ---- LN2 on packed real cols ----
            rows2 = ln_rows(ytile, np_grp, True)
            ynb = sb1.tile([128, 6, 392], BF, tag="ynb", name="ynb")
            ln_apply(ytile, ynb, rows2, np_grp)

            # ---- MLP in two half-passes (halves hT SBUF footprint);
            # half-A accumulates into ytile in place ----
            for half in (0, 1):
                hT = sb1.tile([128, 12, 392], BF, tag="hT", name=f"hT{half}")
                for mi in range(12):
                    mc = 12 * half + mi
                    p1 = ps_mm.tile([128, 512], F32, tag="mm")
                    for kc in range(6):
                        nc.tensor.matmul(p1[:, :np_grp],
                                         w1[:, kc, 128 * mc:128 * (mc + 1)],
                                         ynb[:, kc, :np_grp],
                                         start=(kc == 0), stop=(kc == 5))
                    nc.scalar.activation(out=hT[:, mi, :np_grp],
                                         in_=p1[:, :np_grp],
                                         func=AF.Gelu, bias=b1c[:, mc:mc + 1])
                for oc in range(6):
                    p2 = ps_mm.tile([128, 512], F32, tag="mm")
                    if half == 0:
                        nc.tensor.matmul(p2[:, :np_grp],
                                         b2r[0:1, 128 * oc:128 * (oc + 1)],
                                         ones_row[0:1, :np_grp],
                                         start=True, stop=False)
                    for ki in range(12):
                        kc = 12 * half + ki
                        nc.tensor.matmul(p2[:, :np_grp],
                                         w2[:, kc, 128 * oc:128 * (oc + 1)],
                                         hT[:, ki, :np_grp],
                                         start=(half == 1 and ki == 0),
                                         stop=(ki == 11))
                    if half == 0:
                        nc.vector.tensor_add(out=ytile[:, oc, :np_grp],
                                             in0=ytile[:, oc, :np_grp],
                                             in1=p2[:, :np_grp])
                    else:
                        fo = sb2.tile([128, 392], F32, tag="fo", bufs=1)
                        nc.vector.tensor_add(out=fo[:, :np_grp],
                                             in0=ytile[:, oc, :np_grp],
                                             in1=p2[:, :np_grp])
                        for wi, s in enumerate(grp):
                            nc.sync.dma_start(
                                out=out_d.ap()[128 * oc:128 * (oc + 1),
                                               s * TW:s * TW + nreal],
                                in_=fo[:, offs[wi]:offs[wi] + nreal])

        tinfo = {}
        tinfo[0] = stage1(0)
        next_s1 = 1
        for pi, grp in enumerate(PAIRS):
            t = tinfo.pop(pi)
            nreal = t["nreal"]
            nw = len(grp)
            offs = [wi * nreal for wi in range(nw)]
            np_grp = nw * nreal
            avT = sb1.tile([128, 6, 392], BF, tag="avT", name="avT")
            for wi in range(nw):
                stage2(t, wi, avT)
                if wi == 0 and next_s1 < len(PAIRS):
                    tinfo[next_s1] = stage1(next_s1)
                    next_s1 += 1
            proj_mlp(t, avT, offs, np_grp)

    nc.compile()
    return nc


# ----------------------------------------------------------------------------
# host wrapper
# ----------------------------------------------------------------------------

def _window_assignment():
    interior = [(b, wy, wx) for b in range(B) for wy in range(4) for wx in range(4)]
    right = [(b, wy, 4) for b in range(B) for wy in range(4)]
    bottom = [(b, 4, wx) for b in range(B) for wx in range(4)]
    corner = [(b, 4, 4) for b in range(B)]
    cores = []
    for c in range(NCORES):
        wins = interior[8 * c:8 * c + 8] + right[2 * c:2 * c + 2] \
            + bottom[2 * c:2 * c + 2]
        wins.append(corner[c] if c < 4 else None)
        cores.append(wins)
    return cores


def _prep_consts(ln1_w, ln1_b, qkv_w, qkv_b, proj_w, proj_b, rel_pos_h,
                 rel_pos_w, ln2_w, ln2_b, w1, b1, w2, b2):
    qkv_w = np.asarray(qkv_w, np.float32)
    w1 = np.asarray(w1, np.float32)
    Wq = np.asarray(ln1_w, np.float32)[:, None] * qkv_w
    bq = np.asarray(qkv_b, np.float32) + np.asarray(ln1_b, np.float32) @ qkv_w
    Wq = Wq.copy()
    bq = bq.copy()
    Wq[:, :DIM] *= SCALE
    bq[:DIM] *= SCALE
    W1 = np.asarray(ln2_w, np.float32)[:, None] * w1
    B1 = np.asarray(b1, np.float32) + np.asarray(ln2_b, np.float32) @ w1
    return {
        "wqkv": Wq.reshape(6, 128, 3 * DIM).astype(BF16),
        "bqkv": np.ascontiguousarray(bq.reshape(18, 128).T).astype(np.float32),
        "relhw": np.ascontiguousarray(np.stack(
            [np.concatenate([np.asarray(t, np.float32)[::-1].T] * 2, axis=0)
             for t in (rel_pos_h, rel_pos_w)], axis=1)).astype(BF16),
        "wproj": np.asarray(proj_w, np.float32).reshape(6, 128, DIM).astype(BF16),
        "bproj": np.asarray(proj_b, np.float32).reshape(1, DIM).astype(BF16),
        "w1": W1.reshape(6, 128, MLP_H).astype(BF16),
        "b1": np.ascontiguousarray(B1.reshape(24, 128).T).astype(np.float32),
        "w2": np.asarray(w2, np.float32).reshape(24, 128, DIM).astype(BF16),
        "b2": np.asarray(b2, np.float32).reshape(1, DIM).astype(BF16),
    }


_ORDER_CACHE = {}


def _order_idx(rh, rw):
    key = (rh, rw)
    if key not in _ORDER_CACHE:
        _ORDER_CACHE[key] = np.array(token_order(rh, rw), np.int64)
    return _ORDER_CACHE[key]


def kernel(x, ln1_w, ln1_b, qkv_w, qkv_b, proj_w, proj_b, rel_pos_h,
           rel_pos_w, ln2_w, ln2_b, w1, b1, w2, b2):
    from concourse.bass_utils import run_bass_kernel_spmd

    x = np.asarray(x, np.float32)
    consts = _prep_consts(ln1_w, ln1_b, qkv_w, qkv_b, proj_w, proj_b,
                          rel_pos_h, rel_pos_w, ln2_w, ln2_b, w1, b1, w2, b2)

    if "nc" not in _CACHE:
        _CACHE["nc"] = _build()
    nc = _CACHE["nc"]

    assign = _window_assignment()
    xpad = np.zeros((B, 70, 70, DIM), np.float32)
    xpad[:, :H, :W, :] = x

    in_maps = []
    for c in range(NCORES):
        xwT = np.zeros((DIM, CORE_TOK), np.float32)
        for s, win in enumerate(assign[c]):
            if win is None:
                continue
            b, wy, wx = win
            rh, rw = WCLASSES[s]
            idx = _order_idx(rh, rw)
            blk = xpad[b, 14 * wy:14 * wy + 14, 14 * wx:14 * wx + 14, :]
            xwT[:, s * TW:(s + 1) * TW] = blk[idx[:, 0], idx[:, 1], :].T
        m = {"xwT": xwT.astype(BF16)}
        m.update(consts)
        in_maps.append(m)

    res = run_bass_kernel_spmd(nc, in_maps, core_ids=list(range(NCORES)),
                               **_CACHE.get("run_kwargs", {}))
    _CACHE["last_result"] = res

    out = np.zeros((B, H, W, DIM), np.float32)
    for c in range(NCORES):
        oT = res.results[c]["outT"]
        for s, win in enumerate(assign[c]):
            if win is None:
                continue
            b, wy, wx = win
            rh, rw = WCLASSES[s]
            idx = _order_idx(rh, rw)[:rh * rw]
            out[b, 14 * wy + idx[:, 0], 14 * wx + idx[:, 1], :] = \
                oT[:, s * TW:s * TW + rh * rw].T
    return out


# revision 15
# speedup vs baseline: 1.2420x; 1.0127x over previous
"""Windowed-attention ViT block (SAM-style) on 8 TRN2 NeuronCores.

Feature-major ("^T") layout [dim, tokens] on device. Per core: 13 window
slots processed as 6 same-class PAIRS + 1 solo, so qkv/LN/proj/MLP run at
392-wide free dims. LN stats computed as ones-stationary row matmuls
(avoids fp32 wide-LDW stats matmuls); rstd via Ln+Exp (stays in the
exp table set). Rel-pos handled multiplicatively: exp(S+B) =
exp(S)*EC1*EC2 with EC gathered from exp(q @ relpos_rev) via padded DRAM
copies that make the Toeplitz gather a single 3-dim strided DMA per
(ri, chunk). QK/EP matmuls are issued as head pairs on disjoint 64-row
PE groups; AV is issued col-paired on 64-col groups.
"""
import numpy as np
import ml_dtypes
from contextlib import ExitStack

DIM = 768
NH = 12
HD = 64
WS = 14
H = W = 64
MLP_H = 3072
EPS = 1e-5
SCALE = HD ** -0.5
B = 4
NCORES = 8
TW = WS * WS  # 196
NW_CORE = 13
CORE_TOK = NW_CORE * TW  # 2548

WCLASSES = [(14, 14)] * 8 + [(14, 8)] * 2 + [(8, 14)] * 2 + [(8, 8)]
PAIRS = [(0, 1), (2, 3), (4, 5), (6, 7), (8, 9), (10, 11), (12,)]
REC = 2 * NH * 27  # 648 full token record; per-ri record = 324
HREC = NH * 27     # 324

BF16 = ml_dtypes.bfloat16


def token_order(rh, rw):
    order = [(r, c) for r in range(rh) for c in range(rw)]
    order += [(r, c) for r in range(rh) for c in range(rw, WS)]
    order += [(r, c) for r in range(rh, WS) for c in range(WS)]
    return order


def attn_chunks(rh, rw):
    """Token-offset chunks of the real block: (t0, csz); row-aligned."""
    if rh * rw == 196:
        return [(0, 98), (98, 98)]
    return [(0, rh * rw)]


def col_groups(rh, rw):
    """Key-column groups: (start, n_r2, r2_0, n_c2, c2_0)."""
    g = [(0, rh, 0, rw, 0)]
    if rw < WS:
        g.append((rh * rw, rh, 0, WS - rw, rw))
    if rh < WS:
        g.append((rh * WS, WS - rh, rh, WS, 0))
    return g


_CACHE = {}


def _build():
    import concourse.bass as bass
    import concourse.mybir as mybir
    import concourse.tile as tile
    from concourse import bacc
    from concourse.masks import make_identity

    F32 = mybir.dt.float32
    BF = mybir.dt.bfloat16
    AF = mybir.ActivationFunctionType
    AX = mybir.AxisListType

    classes = WCLASSES
    ncols = NW_CORE * TW

    nc = bacc.Bacc("TRN2", target_bir_lowering=False, debug=False,
                   enable_asserts=False, num_devices=NCORES)

    xwT_d = nc.dram_tensor("xwT", [DIM, ncols], BF, kind="ExternalInput")
    wqkv_d = nc.dram_tensor("wqkv", [6, 128, 3 * DIM], BF, kind="ExternalInput")
    bqkv_d = nc.dram_tensor("bqkv", [128, 18], F32, kind="ExternalInput")
    relhw_d = nc.dram_tensor("relhw", [128, 2, 27], BF, kind="ExternalInput")
    wproj_d = nc.dram_tensor("wproj", [6, 128, DIM], BF, kind="ExternalInput")
    bproj_d = nc.dram_tensor("bproj", [1, DIM], BF, kind="ExternalInput")
    w1_d = nc.dram_tensor("w1", [6, 128, MLP_H], BF, kind="ExternalInput")
    b1_d = nc.dram_tensor("b1", [128, 24], F32, kind="ExternalInput")
    w2_d = nc.dram_tensor("w2", [24, 128, DIM], BF, kind="ExternalInput")
    b2_d = nc.dram_tensor("b2", [1, DIM], BF, kind="ExternalInput")
    out_d = nc.dram_tensor("outT", [DIM, ncols], F32, kind="ExternalOutput")

    with tile.TileContext(nc) as tc, ExitStack() as ctx:
        wp = ctx.enter_context(tc.tile_pool(name="weights", bufs=1))
        sb = ctx.enter_context(tc.tile_pool(name="sb", bufs=2))
        sb1 = ctx.enter_context(tc.tile_pool(name="sb1", bufs=1))
        sb2 = ctx.enter_context(tc.tile_pool(name="sb2", bufs=2))
        ps_mm = ctx.enter_context(tc.tile_pool(name="ps_mm", bufs=3, space="PSUM"))
        ps_at = ctx.enter_context(tc.tile_pool(name="ps_at", bufs=2, space="PSUM"))
        ps_av = ctx.enter_context(tc.tile_pool(name="ps_av", bufs=1, space="PSUM"))
        ps_rw = ctx.enter_context(tc.tile_pool(name="ps_rw", bufs=1, space="PSUM"))
        dr = ctx.enter_context(tc.tile_pool(name="dr", bufs=2, space="DRAM"))

        # ---- constants ----
        wqkv = wp.tile([128, 6, 3 * DIM], BF)
        wproj = wp.tile([128, 6, DIM], BF)
        w1 = wp.tile([128, 6, MLP_H], BF)
        w2 = wp.tile([128, 24, DIM], BF)
        # wqkv on the sync queue (needed first, ahead of window-0 xw);
        # bulky later-phase weights go on the idle scalar queue.
        for kc in range(6):
            nc.sync.dma_start(out=wqkv[:, kc, :], in_=wqkv_d.ap()[kc])
        for kc in range(6):
            nc.scalar.dma_start(out=wproj[:, kc, :], in_=wproj_d.ap()[kc])
            nc.scalar.dma_start(out=w1[:, kc, :], in_=w1_d.ap()[kc])
        for kc in range(24):
            nc.scalar.dma_start(out=w2[:, kc, :], in_=w2_d.ap()[kc])
        bqkv = wp.tile([128, 18], F32)
        nc.sync.dma_start(out=bqkv, in_=bqkv_d.ap())
        relhw = wp.tile([128, 2, 27], BF)
        nc.sync.dma_start(out=relhw, in_=relhw_d.ap())
        bproj = wp.tile([1, DIM], BF)
        nc.sync.dma_start(out=bproj, in_=bproj_d.ap())
        b1c = wp.tile([128, 24], F32)
        nc.sync.dma_start(out=b1c, in_=b1_d.ap())
        b2r = wp.tile([1, DIM], BF)
        nc.sync.dma_start(out=b2r, in_=b2_d.ap())

        ident = wp.tile([128, 128], BF)
        make_identity(nc, ident)
        ones_col = wp.tile([128, 1], F32)
        nc.vector.memset(ones_col, 1.0)
        ones_colb = wp.tile([128, 1], BF)
        nc.vector.memset(ones_colb, 1.0)
        ones_1x128 = wp.tile([1, 128], BF)
        nc.vector.memset(ones_1x128, 1.0)
        ones_row = wp.tile([1, 512], BF)
        nc.vector.memset(ones_row, 1.0)
        eps_c = wp.tile([1, 1], F32)
        nc.vector.memset(eps_c, EPS)

        # warm the PE HAM while initial weight DMAs are in flight
        wu = ps_mm.tile([128, 512], F32, tag="mm", name="warm")
        for _ in range(48):
            nc.tensor.matmul(wu[:, :128], ident, ident, start=True, stop=True)

        def ln_rows(src3, nfree, f32src):
            """LN stats over partition dim via ones-stationary matmuls.

            src3: AP [128, 6, X]. Returns sbuf rows tile [1, 2, nfree]
            bf16 rows (mean, rstd)."""
            sum_ps = ps_rw.tile([1, 512], F32, tag="lnsum", name="lnsum")
            sq_ps = ps_rw.tile([1, 512], F32, tag="lnsq", name="lnsq")
            lnc = ones_col if f32src else ones_colb
            for dc in range(6):
                nc.tensor.matmul(sum_ps[0:1, :nfree], lnc,
                                 src3[:, dc, :nfree],
                                 start=(dc == 0), stop=(dc == 5))
            for dc in range(6):
                sq = sb2.tile([128, 392], BF, tag="sq", bufs=1)
                nc.vector.tensor_mul(out=sq[:, :nfree],
                                     in0=src3[:, dc, :nfree],
                                     in1=src3[:, dc, :nfree])
                nc.tensor.matmul(sq_ps[0:1, :nfree], ones_colb,
                                 sq[:, :nfree],
                                 start=(dc == 0), stop=(dc == 5))
            rows = sb2.tile([1, 2, 392], BF, tag="lnrows")
            # mean = sum/768
            nc.scalar.mul(rows[0:1, 0, :nfree], sum_ps[0:1, :nfree], 1.0 / DIM)
            # var = sq/768 - mean^2 ; rstd = exp(-0.5*ln(var+eps))
            vr = sb2.tile([1, 2, 392], F32, tag="lnvr", bufs=1)
            nc.vector.tensor_mul(out=vr[0:1, 0, :nfree],
                                 in0=rows[0:1, 0, :nfree],
                                 in1=rows[0:1, 0, :nfree])
            nc.scalar.mul(vr[0:1, 1, :nfree], sq_ps[0:1, :nfree], 1.0 / DIM)
            nc.vector.tensor_sub(out=vr[0:1, 1, :nfree],
                                 in0=vr[0:1, 1, :nfree],
                                 in1=vr[0:1, 0, :nfree])
            nc.scalar.activation(out=vr[0:1, 0, :nfree],
                                 in_=vr[0:1, 1, :nfree],
                                 func=AF.Ln, bias=eps_c[0:1])
            nc.scalar.activation(out=rows[0:1, 1, :nfree],
                                 in_=vr[0:1, 0, :nfree],
                                 func=AF.Exp, scale=-0.5)
            return rows

        def ln_apply(src3, dst3, rows, nfree):
            """dst = (src - mean) * rstd, with mean/rstd bcast via matmul."""
            bpm = ps_mm.tile([128, 512], F32, tag="mm", name="lnbm")
            bpr = ps_mm.tile([128, 512], F32, tag="mm", name="lnbr")
            nc.tensor.matmul(bpm[:, :nfree], ones_1x128,
                             rows[0:1, 0, :nfree], start=True, stop=True)
            nc.tensor.matmul(bpr[:, :nfree], ones_1x128,
                             rows[0:1, 1, :nfree], start=True, stop=True)
            for dc in range(6):
                tscr = sb2.tile([128, 392], F32, tag="tscr", bufs=1)
                nc.vector.tensor_sub(out=tscr[:, :nfree],
                                     in0=src3[:, dc, :nfree],
                                     in1=bpm[:, :nfree])
                nc.vector.tensor_mul(out=dst3[:, dc, :nfree],
                                     in0=tscr[:, :nfree],
                                     in1=bpr[:, :nfree])

        def stage1(pi, res):
            """Load + LN1 + qkv for a pair; EP/epd/gather per window.

            Generator: yields between work quanta so the driver can weave
            independent instruction streams (keeps the PE queue free of
            head-of-line stalls and the HAM warm). Fills `res` dict."""
            grp = PAIRS[pi]
            nw = len(grp)
            nfree = nw * TW
            tok0 = grp[0] * TW
            rh, rw = classes[grp[0]]
            nreal = rh * rw
            chunks = attn_chunks(rh, rw)

            # ---- load xw^T (pair-wide) ----
            xw = sb.tile([128, 6, 392], BF, tag="xw")
            for dc in range(6):
                nc.sync.dma_start(
                    out=xw[:, dc, :nfree],
                    in_=xwT_d.ap()[128 * dc:128 * (dc + 1), tok0:tok0 + nfree])

            yield
            # ---- LN1 (full cols so pads normalize to exact 0) ----
            rows1 = ln_rows(xw, nfree, False)
            yield
            xs = sb.tile([128, 6, 392], BF, tag="xs", bufs=1)
            ln_apply(xw, xs, rows1, nfree)
            yield

            # ---- qkv^T (pair-wide) ----
            qkvT = sb.tile([128, 18, 392], BF, tag="qkvT")
            for mc in range(18):
                qp = ps_mm.tile([128, 512], F32, tag="mm")
                for kc in range(6):
                    nc.tensor.matmul(qp[:, :nfree],
                                     wqkv[:, kc, 128 * mc:128 * (mc + 1)],
                                     xs[:, kc, :nfree],
                                     start=(kc == 0), stop=(kc == 5))
                nc.scalar.activation(out=qkvT[:, mc, :nfree],
                                     in_=qp[:, :nfree],
                                     func=AF.Identity,
                                     bias=bqkv[:, mc:mc + 1])
                if mc % 2 == 1:
                    yield

            # ---- rel-pos EP + padded epd + gather, per window ----
            ec = {}
            for wi, s in enumerate(grp):
                woff = wi * TW
                # EP[q, ri, h, s27] = exp(q . relpos_rev), head-pair packed
                etks = []
                for ci, (t0, csz) in enumerate(chunks):
                    etk = sb.tile([128, 2, NH, 27], BF, tag=f"etk{wi}{ci}",
                                  name=f"etk{wi}{ci}", bufs=1)
                    etks.append(etk)
                    for hp in range(6):
                        pps = []
                        for par in range(2):
                            po = 64 * par
                            pp = ps_mm.tile([128, 512], F32, tag="mm")
                            nc.tensor.matmul(
                                pp[:csz, :54],
                                qkvT[po:po + 64, hp, woff + t0:woff + t0 + csz],
                                relhw[po:po + 64, :, :],
                                start=True, stop=True)
                            pps.append(pp)
                        for par in range(2):
                            nc.scalar.activation(
                                out=etk[:csz, :, 2 * hp + par, :],
                                in_=pps[par][:csz, :54].rearrange(
                                    "p (r d) -> p r d", r=2),
                                func=AF.Exp)
                        if hp % 2 == 1:
                            yield
                # Padded DRAM copies make the Toeplitz gather a uniform
                # strided read: ri0 places token records at tok*324 + row
                # (row stride rw*324+1), so record + (13 - row + kh) is
                # linear at stride 324. ri1 places records at tok*325 + j
                # (j = in-row pos; within-row stride 326, row stride
                # rw*325), so record + (13 - j + kw) is linear at 325.
                nrows_all = (TW if rh * rw == 196 else nreal) // rw
                ntok = nrows_all * rw
                sz0 = nrows_all * (rw * HREC + 1) + 350
                sz1 = ntok * 325 + 350
                epd0 = dr.tile([sz0], BF, tag=f"epd0_{wi}",
                               name=f"epd0_{wi}")
                epd1 = dr.tile([sz1], BF, tag=f"epd1_{wi}",
                               name=f"epd1_{wi}")
                b0 = epd0[:]
                b1 = epd1[:]
                for ci, (t0, csz) in enumerate(chunks):
                    nr = csz // rw
                    r0 = t0 // rw
                    dst0 = bass.AP(
                        tensor=b0.tensor,
                        offset=b0.offset + r0 * (rw * HREC + 1),
                        ap=[[rw * HREC + 1, nr], [HREC, rw], [1, HREC]])
                    nc.gpsimd.dma_start(out=dst0,
                                        in_=etks[ci][:csz, 0, :, :])
                    dst1 = bass.AP(
                        tensor=b1.tensor,
                        offset=b1.offset + t0 * 325,
                        ap=[[rw * 325, nr], [326, rw], [1, HREC]])
                    nc.gpsimd.dma_start(out=dst1,
                                        in_=etks[ci][:csz, 1, :, :])
                for ci, (t0, csz) in enumerate(chunks):
                    e0 = sb.tile([128, NH, 14], BF, tag=f"ec0{ci}{wi}",
                                 name=f"ec0{ci}{wi}")
                    src0 = bass.AP(
                        tensor=b0.tensor, offset=b0.offset + t0 * HREC + 13,
                        ap=[[HREC, csz], [27, NH], [1, 14]])
                    nc.sync.dma_start(out=e0[:csz, :, :], in_=src0)
                    e1 = sb.tile([128, NH, 14], BF, tag=f"ec1{ci}{wi}",
                                 name=f"ec1{ci}{wi}")
                    src1 = bass.AP(
                        tensor=b1.tensor, offset=b1.offset + t0 * 325 + 13,
                        ap=[[325, csz], [27, NH], [1, 14]])
                    nc.scalar.dma_start(out=e1[:csz, :, :], in_=src1)
                    ec[(0, ci, wi)] = e0
                    ec[(1, ci, wi)] = e1
                yield

            res.update(dict(xw=xw, qkvT=qkvT, ec=ec, chunks=chunks, rh=rh,
                            rw=rw, nreal=nreal, grp=grp, nfree=nfree))

        def stage2(t, wi, avT):
            """Attention for window wi of pair t -> avT[:, :, wi*TW...].

            Generator; yields between hp quanta."""
            rh = t["rh"]; rw = t["rw"]; nreal = t["nreal"]
            chunks = t["chunks"]; qkvT = t["qkvT"]; ec = t["ec"]
            woff = wi * TW
            cg = col_groups(rh, rw)
            for hp in range(6):
                # v^T tiles (both heads of the pair)
                vt = []
                for (j0, jn) in ((0, 128), (128, 68)):
                    pv = ps_at.tile([128, 2, 196], BF, tag="at", name="vtr")
                    nc.tensor.transpose(pv[:jn, 0, :128],
                                        qkvT[:, 12 + hp, woff + j0:woff + j0 + jn],
                                        ident)
                    vtk = sb.tile([128, 128], BF, tag=f"vtk{j0}")
                    nc.vector.tensor_copy(out=vtk[:jn, :], in_=pv[:jn, 0, :128])
                    vt.append((j0, jn, vtk))
                # scores for BOTH heads first, hh-inner so the two
                # row-group mms (lhsT base partition 0 vs 64) run
                # concurrently on the PE
                sps = [ps_at.tile([128, 2, 196], F32, tag="at", name="sp0"),
                       ps_at.tile([128, 2, 196], F32, tag="at", name="sp1")]
                for ci, (t0, csz) in enumerate(chunks):
                    for hh in (0, 1):
                        po = 64 * hh
                        nc.tensor.matmul(
                            sps[hh][:csz, ci, :],
                            qkvT[po:po + 64, hp, woff + t0:woff + t0 + csz],
                            qkvT[po:po + 64, 6 + hp, woff:woff + TW],
                            start=True, stop=True)
                yield
                ET = {}
                for hh in (0, 1):
                    h = 2 * hp + hh
                    sp = sps[hh]
                    E = sb.tile([128, 2, 196], BF, tag="E", name="E")
                    zt = sb2.tile([128, 4], F32, tag="z")
                    for ci, (t0, csz) in enumerate(chunks):
                        nc.scalar.activation(out=E[:csz, ci, :],
                                             in_=sp[:csz, ci, :],
                                             func=AF.Exp)
                        for (cst, nr2, r20, nc2, c20) in cg:
                            ev = E[:csz, ci, cst:cst + nr2 * nc2].rearrange(
                                "p (a b) -> p a b", a=nr2)
                            nc.vector.tensor_mul(
                                out=ev, in0=ev,
                                in1=ec[(0, ci, wi)][:csz, h, r20:r20 + nr2]
                                [:, :, None].broadcast_to([csz, nr2, nc2]))
                            nc.vector.tensor_mul(
                                out=ev, in0=ev,
                                in1=ec[(1, ci, wi)][:csz, h, c20:c20 + nc2]
                                [:, None, :].broadcast_to([csz, nr2, nc2]))
                        nc.vector.reduce_sum(out=zt[:csz, ci:ci + 1],
                                             in_=E[:csz, ci, :], axis=AX.X)
                    nch = len(chunks)
                    csz0 = chunks[0][1]
                    nc.vector.reciprocal(out=zt[:csz0, 2:2 + nch],
                                         in_=zt[:csz0, 0:nch])
                    ETt = [sb.tile([128, 196], BF, tag="ET0", name="ET0"),
                           sb.tile([128, 196], BF, tag="ET1", name="ET1")]
                    ET[hh] = ETt
                    for ci, (t0, csz) in enumerate(chunks):
                        nc.vector.tensor_scalar_mul(
                            out=E[:csz, ci, :], in0=E[:csz, ci, :],
                            scalar1=zt[:csz, 2 + ci:3 + ci])
                        for ji, (j0, jn) in enumerate(((0, 128), (128, 68))):
                            pe = ps_at.tile([128, 2, 196], BF, tag="at",
                                            name="pe")
                            nc.tensor.transpose(pe[:jn, 0, :csz],
                                                E[:csz, ci, j0:j0 + jn],
                                                ident[:csz, :csz])
                            nc.vector.tensor_copy(
                                out=ETt[ji][:jn, t0:t0 + csz],
                                in_=pe[:jn, 0, :csz])
                    yield
                # AV, col-paired across hh
                avp = ps_av.tile([128, 196], F32, tag="av")
                for ji, (j0, jn, vtk) in enumerate(vt):
                    for hh in (0, 1):
                        nc.tensor.matmul(
                            avp[64 * hh:64 * hh + 64, :nreal],
                            vtk[:jn, 64 * hh:64 * hh + 64],
                            ET[hh][ji][:jn, :nreal],
                            start=(ji == 0), stop=(ji == 1),
                            tile_position=(0, 64 * hh))
                nc.vector.tensor_copy(out=avT[:, hp, woff:woff + nreal],
                                      in_=avp[:, :nreal])
                yield

        def proj_ln2(t, avT, offs, np_grp, out):
            """proj + residual + LN2 for the pair, packed cols. Generator."""
            grp = t["grp"]; xw = t["xw"]; nreal = t["nreal"]
            nw = len(grp)
            span = (nw - 1) * TW + nreal  # proj free span incl. slot gap
            ytile = sb1.tile([128, 6, 392], F32, tag="y", name="y")
            for oc in range(6):
                zp = ps_mm.tile([128, 512], F32, tag="mm")
                nc.tensor.matmul(zp[:, :span],
                                 bproj[0:1, 128 * oc:128 * (oc + 1)],
                                 ones_row[0:1, :span],
                                 start=True, stop=False)
                for kc in range(6):
                    nc.tensor.matmul(zp[:, :span],
                                     wproj[:, kc, 128 * oc:128 * (oc + 1)],
                                     avT[:, kc, :span],
                                     start=False, stop=(kc == 5))
                for wi in range(nw):
                    nc.vector.tensor_add(
                        out=ytile[:, oc, offs[wi]:offs[wi] + nreal],
                        in0=xw[:, oc, wi * TW:wi * TW + nreal],
                        in1=zp[:, wi * TW:wi * TW + nreal])
                if oc % 2 == 1:
                    yield

            # ---- LN2 on packed real cols ----
            rows2 = ln_rows(ytile, np_grp, True)
            yield
            ynb = sb1.tile([128, 6, 392], BF, tag="ynb", name="ynb")
            ln_apply(ytile, ynb, rows2, np_grp)
            yield
            out["ytile"] = ytile
            out["ynb"] = ynb

        def mlp(t, ytile, ynb, offs, np_grp):
            """MLP for the pair in two half-passes (halves hT footprint);
            half-A accumulates into ytile in place. Generator."""
            grp = t["grp"]; nreal = t["nreal"]
            for half in (0, 1):
                hT = sb1.tile([128, 12, 392], BF, tag="hT", name=f"hT{half}")
                for mi in range(12):
                    mc = 12 * half + mi
                    p1 = ps_mm.tile([128, 512], F32, tag="mm")
                    for kc in range(6):
                        nc.tensor.matmul(p1[:, :np_grp],
                                         w1[:, kc, 128 * mc:128 * (mc + 1)],
                                         ynb[:, kc, :np_grp],
                                         start=(kc == 0), stop=(kc == 5))
                    nc.scalar.activation(out=hT[:, mi, :np_grp],
                                         in_=p1[:, :np_grp],
                                         func=AF.Gelu, bias=b1c[:, mc:mc + 1])
                    if mi % 2 == 1:
                        yield
                for oc in range(6):
                    p2 = ps_mm.tile([128, 512], F32, tag="mm")
                    if half == 0:
                        nc.tensor.matmul(p2[:, :np_grp],
                                         b2r[0:1, 128 * oc:128 * (oc + 1)],
                                         ones_row[0:1, :np_grp],
                                         start=True, stop=False)
                    for ki in range(12):
                        kc = 12 * half + ki
                        nc.tensor.matmul(p2[:, :np_grp],
                                         w2[:, kc, 128 * oc:128 * (oc + 1)],
                                         hT[:, ki, :np_grp],
                                         start=(half == 1 and ki == 0),
                                         stop=(ki == 11))
                    if half == 0:
                        nc.vector.tensor_add(out=ytile[:, oc, :np_grp],
                                             in0=ytile[:, oc, :np_grp],
                                             in1=p2[:, :np_grp])
                    else:
                        fo = sb2.tile([128, 392], F32, tag="fo", bufs=1)
                        nc.vector.tensor_add(out=fo[:, :np_grp],
                                             in0=ytile[:, oc, :np_grp],
                                             in1=p2[:, :np_grp])
                        for wi, s in enumerate(grp):
                            nc.sync.dma_start(
                                out=out_d.ap()[128 * oc:128 * (oc + 1),
                                               s * TW:s * TW + nreal],
                                in_=fo[:, offs[wi]:offs[wi] + nreal])
                    yield

        def drain(*gens):
            """Round-robin the generators until all are exhausted."""
            live = [g for g in gens if g is not None]
            while live:
                nxt = []
                for g in live:
                    try:
                        next(g)
                        nxt.append(g)
                    except StopIteration:
                        pass
                live = nxt

        def attn_gen(t, avT, offs, np_grp, out):
            for wi in range(len(t["grp"])):
                yield from stage2(t, wi, avT)
            yield from proj_ln2(t, avT, offs, np_grp, out)

        tinfo = [dict() for _ in PAIRS]
        drain(stage1(0, tinfo[0]))
        mlp_gen = None
        for pi, grp in enumerate(PAIRS):
            t = tinfo[pi]
            nreal = t["nreal"]
            nw = len(grp)
            offs = [wi * nreal for wi in range(nw)]
            np_grp = nw * nreal
            avT = sb1.tile([128, 6, 392], BF, tag="avT", name="avT")
            out = {}
            s1_gen = stage1(pi + 1, tinfo[pi + 1]) \
                if pi + 1 < len(PAIRS) else None
            drain(attn_gen(t, avT, offs, np_grp, out), s1_gen, mlp_gen)
            mlp_gen = mlp(t, out["ytile"], out["ynb"], offs, np_grp)
        drain(mlp_gen)

    nc.compile()
    return nc


# ----------------------------------------------------------------------------
# host wrapper
# ----------------------------------------------------------------------------

def _window_assignment():
    interior = [(b, wy, wx) for b in range(B) for wy in range(4) for wx in range(4)]
    right = [(b, wy, 4) for b in range(B) for wy in range(4)]
    bottom = [(b, 4, wx) for b in range(B) for wx in range(4)]
    corner = [(b, 4, 4) for b in range(B)]
    cores = []
    for c in range(NCORES):
        wins = interior[8 * c:8 * c + 8] + right[2 * c:2 * c + 2] \
            + bottom[2 * c:2 * c + 2]
        wins.append(corner[c] if c < 4 else None)
        cores.append(wins)
    return cores


def _prep_consts(ln1_w, ln1_b, qkv_w, qkv_b, proj_w, proj_b, rel_pos_h,
                 rel_pos_w, ln2_w, ln2_b, w1, b1, w2, b2):
    qkv_w = np.asarray(qkv_w, np.float32)
    w1 = np.asarray(w1, np.float32)
    Wq = np.asarray(ln1_w, np.float32)[:, None] * qkv_w
    bq = np.asarray(qkv_b, np.float32) + np.asarray(ln1_b, np.float32) @ qkv_w
    Wq = Wq.copy()
    bq = bq.copy()
    Wq[:, :DIM] *= SCALE
    bq[:DIM] *= SCALE
    W1 = np.asarray(ln2_w, np.float32)[:, None] * w1
    B1 = np.asarray(b1, np.float32) + np.asarray(ln2_b, np.float32) @ w1
    return {
        "wqkv": Wq.reshape(6, 128, 3 * DIM).astype(BF16),
        "bqkv": np.ascontiguousarray(bq.reshape(18, 128).T).astype(np.float32),
        "relhw": np.ascontiguousarray(np.stack(
            [np.concatenate([np.asarray(t, np.float32)[::-1].T] * 2, axis=0)
             for t in (rel_pos_h, rel_pos_w)], axis=1)).astype(BF16),
        "wproj": np.asarray(proj_w, np.float32).reshape(6, 128, DIM).astype(BF16),
        "bproj": np.asarray(proj_b, np.float32).reshape(1, DIM).astype(BF16),
        "w1": W1.reshape(6, 128, MLP_H).astype(BF16),
        "b1": np.ascontiguousarray(B1.reshape(24, 128).T).astype(np.float32),
        "w2": np.asarray(w2, np.float32).reshape(24, 128, DIM).astype(BF16),
        "b2": np.asarray(b2, np.float32).reshape(1, DIM).astype(BF16),
    }


_ORDER_CACHE = {}


def _order_idx(rh, rw):
    key = (rh, rw)
    if key not in _ORDER_CACHE:
        _ORDER_CACHE[key] = np.array(token_order(rh, rw), np.int64)
    return _ORDER_CACHE[key]


def kernel(x, ln1_w, ln1_b, qkv_w, qkv_b, proj_w, proj_b, rel_pos_h,
           rel_pos_w, ln2_w, ln2_b, w1, b1, w2, b2):
    from concourse.bass_utils import run_bass_kernel_spmd

    x = np.asarray(x, np.float32)
    consts = _prep_consts(ln1_w, ln1_b, qkv_w, qkv_b, proj_w, proj_b,
                          rel_pos_h, rel_pos_w, ln2_w, ln2_b, w1, b1, w2, b2)

    if "nc" not in _CACHE:
        _CACHE["nc"] = _build()
    nc = _CACHE["nc"]

    assign = _window_assignment()
    xpad = np.zeros((B, 70, 70, DIM), np.float32)
    xpad[:, :H, :W, :] = x

    in_maps = []
    for c in range(NCORES):
        xwT = np.zeros((DIM, CORE_TOK), np.float32)
        for s, win in enumerate(assign[c]):
            if win is None:
                continue
            b, wy, wx = win
            rh, rw = WCLASSES[s]
            idx = _order_idx(rh, rw)
            blk = xpad[b, 14 * wy:14 * wy + 14, 14 * wx:14 * wx + 14, :]
            xwT[:, s * TW:(s + 1) * TW] = blk[idx[:, 0], idx[:, 1], :].T
        m = {"xwT": xwT.astype(BF16)}
        m.update(consts)
        in_maps.append(m)

    res = run_bass_kernel_spmd(nc, in_maps, core_ids=list(range(NCORES)),
                               **_CACHE.get("run_kwargs", {}))
    _CACHE["last_result"] = res

    out = np.zeros((B, H, W, DIM), np.float32)
    for c in range(NCORES):
        oT = res.results[c]["outT"]
        for s, win in enumerate(assign[c]):
            if win is None:
                continue
            b, wy, wx = win
            rh, rw = WCLASSES[s]
            idx = _order_idx(rh, rw)[:rh * rw]
            out[b, 14 * wy + idx[:, 0], 14 * wx + idx[:, 1], :] = \
                oT[:, s * TW:s * TW + rh * rw].T
    return out


# revision 19
# speedup vs baseline: 1.4174x; 1.1412x over previous
"""Windowed-attention ViT block (SAM-style) on 8 TRN2 NeuronCores.

Feature-major ("^T") layout [dim, tokens] on device. Per core: 13 window
slots processed as 6 same-class PAIRS + 1 solo, so qkv/LN/proj/MLP run at
392-wide free dims. LN stats computed as ones-stationary row matmuls
(avoids fp32 wide-LDW stats matmuls); rstd via Ln+Exp (stays in the
exp table set). Rel-pos handled multiplicatively: exp(S+B) =
exp(S)*EC1*EC2 with EC gathered from exp(q @ relpos_rev) via padded DRAM
copies that make the Toeplitz gather a single 3-dim strided DMA per
(ri, chunk). QK/EP matmuls are issued as head pairs on disjoint 64-row
PE groups; AV is issued col-paired on 64-col groups.
"""
import numpy as np
import ml_dtypes
from contextlib import ExitStack

DIM = 768
NH = 12
HD = 64
WS = 14
H = W = 64
MLP_H = 3072
EPS = 1e-5
SCALE = HD ** -0.5
B = 4
NCORES = 8
TW = WS * WS  # 196
NW_CORE = 13
CORE_TOK = NW_CORE * TW  # 2548

WCLASSES = [(14, 14)] * 8 + [(14, 8)] * 2 + [(8, 14)] * 2 + [(8, 8)]
PAIRS = [(0, 1), (2, 3), (4, 5), (6, 7), (8, 9), (10, 11), (12,)]
REC = 2 * NH * 27  # 648 full token record; per-ri record = 324
HREC = NH * 27     # 324

BF16 = ml_dtypes.bfloat16


def token_order(rh, rw):
    order = [(r, c) for r in range(rh) for c in range(rw)]
    order += [(r, c) for r in range(rh) for c in range(rw, WS)]
    order += [(r, c) for r in range(rh, WS) for c in range(WS)]
    return order


def attn_chunks(rh, rw):
    """Token-offset chunks of the real block: (t0, csz); row-aligned."""
    if rh * rw == 196:
        return [(0, 98), (98, 98)]
    return [(0, rh * rw)]


def col_groups(rh, rw):
    """Key-column groups: (start, n_r2, r2_0, n_c2, c2_0)."""
    g = [(0, rh, 0, rw, 0)]
    if rw < WS:
        g.append((rh * rw, rh, 0, WS - rw, rw))
    if rh < WS:
        g.append((rh * WS, WS - rh, rh, WS, 0))
    return g


_CACHE = {}


def _build():
    import concourse.bass as bass
    import concourse.mybir as mybir
    import concourse.tile as tile
    from concourse import bacc
    from concourse.masks import make_identity

    F32 = mybir.dt.float32
    BF = mybir.dt.bfloat16
    AF = mybir.ActivationFunctionType
    AX = mybir.AxisListType

    classes = WCLASSES
    ncols = NW_CORE * TW

    nc = bacc.Bacc("TRN2", target_bir_lowering=False, debug=False,
                   enable_asserts=False, num_devices=NCORES)

    xwT_d = nc.dram_tensor("xwT", [DIM, ncols], BF, kind="ExternalInput")
    wqkv_d = nc.dram_tensor("wqkv", [6, 128, 3 * DIM], BF, kind="ExternalInput")
    bqkv_d = nc.dram_tensor("bqkv", [128, 18], F32, kind="ExternalInput")
    relhw_d = nc.dram_tensor("relhw", [128, 2, 27], BF, kind="ExternalInput")
    wproj_d = nc.dram_tensor("wproj", [6, 128, DIM], BF, kind="ExternalInput")
    bproj_d = nc.dram_tensor("bproj", [1, DIM], BF, kind="ExternalInput")
    w1_d = nc.dram_tensor("w1", [6, 128, MLP_H], BF, kind="ExternalInput")
    b1_d = nc.dram_tensor("b1", [128, 24], F32, kind="ExternalInput")
    w2_d = nc.dram_tensor("w2", [24, 128, DIM], BF, kind="ExternalInput")
    b2_d = nc.dram_tensor("b2", [1, DIM], BF, kind="ExternalInput")
    out_d = nc.dram_tensor("outT", [DIM, ncols], F32, kind="ExternalOutput")

    with tile.TileContext(nc) as tc, ExitStack() as ctx:
        wp = ctx.enter_context(tc.tile_pool(name="weights", bufs=1))
        sb = ctx.enter_context(tc.tile_pool(name="sb", bufs=2))
        sb1 = ctx.enter_context(tc.tile_pool(name="sb1", bufs=1))
        sb2 = ctx.enter_context(tc.tile_pool(name="sb2", bufs=2))
        ps_mm = ctx.enter_context(tc.tile_pool(name="ps_mm", bufs=3, space="PSUM"))
        ps_at = ctx.enter_context(tc.tile_pool(name="ps_at", bufs=2, space="PSUM"))
        ps_av = ctx.enter_context(tc.tile_pool(name="ps_av", bufs=1, space="PSUM"))
        ps_rw = ctx.enter_context(tc.tile_pool(name="ps_rw", bufs=1, space="PSUM"))
        dr = ctx.enter_context(tc.tile_pool(name="dr", bufs=2, space="DRAM"))

        # ---- constants ----
        wqkv = wp.tile([128, 6, 3 * DIM], BF)
        wproj = wp.tile([128, 6, DIM], BF)
        w1 = wp.tile([128, 6, MLP_H], BF)
        w2 = wp.tile([128, 24, DIM], BF)
        # wqkv on the sync queue (needed first, ahead of window-0 xw);
        # bulky later-phase weights go on the idle scalar queue.
        for kc in range(6):
            nc.sync.dma_start(out=wqkv[:, kc, :], in_=wqkv_d.ap()[kc])
        for kc in range(6):
            nc.scalar.dma_start(out=wproj[:, kc, :], in_=wproj_d.ap()[kc])
            nc.scalar.dma_start(out=w1[:, kc, :], in_=w1_d.ap()[kc])
        for kc in range(24):
            nc.scalar.dma_start(out=w2[:, kc, :], in_=w2_d.ap()[kc])
        bqkv = wp.tile([128, 18], F32)
        nc.sync.dma_start(out=bqkv, in_=bqkv_d.ap())
        relhw = wp.tile([128, 2, 27], BF)
        nc.sync.dma_start(out=relhw, in_=relhw_d.ap())
        bproj = wp.tile([1, DIM], BF)
        nc.sync.dma_start(out=bproj, in_=bproj_d.ap())
        b1c = wp.tile([128, 24], F32)
        nc.sync.dma_start(out=b1c, in_=b1_d.ap())
        b2r = wp.tile([1, DIM], BF)
        nc.sync.dma_start(out=b2r, in_=b2_d.ap())

        ident = wp.tile([128, 128], BF)
        make_identity(nc, ident)
        ones_col = wp.tile([128, 1], F32)
        nc.vector.memset(ones_col, 1.0)
        ones_colb = wp.tile([128, 1], BF)
        nc.vector.memset(ones_colb, 1.0)
        ones_1x128 = wp.tile([1, 128], BF)
        nc.vector.memset(ones_1x128, 1.0)
        ones_row = wp.tile([1, 512], BF)
        nc.vector.memset(ones_row, 1.0)
        eps_c = wp.tile([1, 1], F32)
        nc.vector.memset(eps_c, EPS)

        # warm the PE HAM while initial weight DMAs are in flight
        wu = ps_mm.tile([128, 512], F32, tag="mm", name="warm")
        for _ in range(48):
            nc.tensor.matmul(wu[:, :128], ident, ident, start=True, stop=True)

        def ln_rows(src3, nfree, f32src):
            """LN stats over partition dim via ones-stationary matmuls.

            src3: AP [128, 6, X]. Returns sbuf rows tile [1, 2, nfree]
            bf16 rows (mean, rstd)."""
            sum_ps = ps_rw.tile([1, 512], F32, tag="lnsum", name="lnsum")
            sq_ps = ps_rw.tile([1, 512], F32, tag="lnsq", name="lnsq")
            lnc = ones_col if f32src else ones_colb
            for dc in range(6):
                nc.tensor.matmul(sum_ps[0:1, :nfree], lnc,
                                 src3[:, dc, :nfree],
                                 start=(dc == 0), stop=(dc == 5))
            for dc in range(6):
                sq = sb2.tile([128, 392], BF, tag="sq", bufs=1)
                nc.vector.tensor_mul(out=sq[:, :nfree],
                                     in0=src3[:, dc, :nfree],
                                     in1=src3[:, dc, :nfree])
                nc.tensor.matmul(sq_ps[0:1, :nfree], ones_colb,
                                 sq[:, :nfree],
                                 start=(dc == 0), stop=(dc == 5))
            rows = sb2.tile([1, 2, 392], BF, tag="lnrows")
            # mean = sum/768
            nc.scalar.mul(rows[0:1, 0, :nfree], sum_ps[0:1, :nfree], 1.0 / DIM)
            # var = sq/768 - mean^2 ; rstd = exp(-0.5*ln(var+eps))
            vr = sb2.tile([1, 2, 392], F32, tag="lnvr", bufs=1)
            nc.vector.tensor_mul(out=vr[0:1, 0, :nfree],
                                 in0=rows[0:1, 0, :nfree],
                                 in1=rows[0:1, 0, :nfree])
            nc.scalar.mul(vr[0:1, 1, :nfree], sq_ps[0:1, :nfree], 1.0 / DIM)
            nc.vector.tensor_sub(out=vr[0:1, 1, :nfree],
                                 in0=vr[0:1, 1, :nfree],
                                 in1=vr[0:1, 0, :nfree])
            nc.scalar.activation(out=vr[0:1, 0, :nfree],
                                 in_=vr[0:1, 1, :nfree],
                                 func=AF.Ln, bias=eps_c[0:1])
            nc.scalar.activation(out=rows[0:1, 1, :nfree],
                                 in_=vr[0:1, 0, :nfree],
                                 func=AF.Exp, scale=-0.5)
            return rows

        def ln_apply(src3, dst3, rows, nfree):
            """dst = (src - mean) * rstd, with mean/rstd bcast via matmul."""
            bpm = ps_mm.tile([128, 512], F32, tag="mm", name="lnbm")
            bpr = ps_mm.tile([128, 512], F32, tag="mm", name="lnbr")
            nc.tensor.matmul(bpm[:, :nfree], ones_1x128,
                             rows[0:1, 0, :nfree], start=True, stop=True)
            nc.tensor.matmul(bpr[:, :nfree], ones_1x128,
                             rows[0:1, 1, :nfree], start=True, stop=True)
            for dc in range(6):
                tscr = sb2.tile([128, 392], F32, tag="tscr", bufs=1)
                nc.vector.tensor_sub(out=tscr[:, :nfree],
                                     in0=src3[:, dc, :nfree],
                                     in1=bpm[:, :nfree])
                nc.vector.tensor_mul(out=dst3[:, dc, :nfree],
                                     in0=tscr[:, :nfree],
                                     in1=bpr[:, :nfree])

        def stage1(pi, res):
            """Load + LN1 + qkv for a pair; EP/epd/gather per window.

            Generator: yields between work quanta so the driver can weave
            independent instruction streams (keeps the PE queue free of
            head-of-line stalls and the HAM warm). Fills `res` dict."""
            grp = PAIRS[pi]
            nw = len(grp)
            nfree = nw * TW
            tok0 = grp[0] * TW
            rh, rw = classes[grp[0]]
            nreal = rh * rw
            chunks = attn_chunks(rh, rw)

            # ---- load xw^T (pair-wide) ----
            xw = sb.tile([128, 6, 392], BF, tag="xw")
            for dc in range(6):
                nc.sync.dma_start(
                    out=xw[:, dc, :nfree],
                    in_=xwT_d.ap()[128 * dc:128 * (dc + 1), tok0:tok0 + nfree])

            yield
            # ---- LN1 (full cols so pads normalize to exact 0) ----
            rows1 = ln_rows(xw, nfree, False)
            yield
            xs = sb.tile([128, 6, 392], BF, tag="xs", bufs=1)
            ln_apply(xw, xs, rows1, nfree)
            yield

            # ---- qkv^T (pair-wide) ----
            qkvT = sb.tile([128, 18, 392], BF, tag="qkvT")
            for mc in range(18):
                qp = ps_mm.tile([128, 512], F32, tag="mm")
                for kc in range(6):
                    nc.tensor.matmul(qp[:, :nfree],
                                     wqkv[:, kc, 128 * mc:128 * (mc + 1)],
                                     xs[:, kc, :nfree],
                                     start=(kc == 0), stop=(kc == 5))
                nc.scalar.activation(out=qkvT[:, mc, :nfree],
                                     in_=qp[:, :nfree],
                                     func=AF.Identity,
                                     bias=bqkv[:, mc:mc + 1])
                if mc % 2 == 1:
                    yield

            # ---- rel-pos EP + padded epd + gather, per window ----
            ec = {}
            for wi, s in enumerate(grp):
                woff = wi * TW
                # EP[q, ri, h, s27] = exp(q . relpos_rev), head-pair packed
                etks = []
                for ci, (t0, csz) in enumerate(chunks):
                    etk = sb.tile([128, 2, NH, 27], BF, tag=f"etk{wi}{ci}",
                                  name=f"etk{wi}{ci}", bufs=1)
                    etks.append(etk)
                    for hp in range(6):
                        pps = []
                        for par in range(2):
                            po = 64 * par
                            pp = ps_mm.tile([128, 512], F32, tag="mm")
                            nc.tensor.matmul(
                                pp[:csz, :54],
                                qkvT[po:po + 64, hp, woff + t0:woff + t0 + csz],
                                relhw[po:po + 64, :, :],
                                start=True, stop=True)
                            pps.append(pp)
                        for par in range(2):
                            nc.scalar.activation(
                                out=etk[:csz, :, 2 * hp + par, :],
                                in_=pps[par][:csz, :54].rearrange(
                                    "p (r d) -> p r d", r=2),
                                func=AF.Exp)
                        if hp % 2 == 1:
                            yield
                # Padded DRAM copies make the Toeplitz gather a uniform
                # strided read: ri0 places token records at tok*324 + row
                # (row stride rw*324+1), so record + (13 - row + kh) is
                # linear at stride 324. ri1 places records at tok*325 + j
                # (j = in-row pos; within-row stride 326, row stride
                # rw*325), so record + (13 - j + kw) is linear at 325.
                nrows_all = (TW if rh * rw == 196 else nreal) // rw
                ntok = nrows_all * rw
                sz0 = nrows_all * (rw * HREC + 1) + 350
                sz1 = ntok * 325 + 350
                epd0 = dr.tile([sz0], BF, tag=f"epd0_{wi}",
                               name=f"epd0_{wi}")
                epd1 = dr.tile([sz1], BF, tag=f"epd1_{wi}",
                               name=f"epd1_{wi}")
                b0 = epd0[:]
                b1 = epd1[:]
                for ci, (t0, csz) in enumerate(chunks):
                    nr = csz // rw
                    r0 = t0 // rw
                    dst0 = bass.AP(
                        tensor=b0.tensor,
                        offset=b0.offset + r0 * (rw * HREC + 1),
                        ap=[[rw * HREC + 1, nr], [HREC, rw], [1, HREC]])
                    nc.gpsimd.dma_start(out=dst0,
                                        in_=etks[ci][:csz, 0, :, :])
                    dst1 = bass.AP(
                        tensor=b1.tensor,
                        offset=b1.offset + t0 * 325,
                        ap=[[rw * 325, nr], [326, rw], [1, HREC]])
                    nc.gpsimd.dma_start(out=dst1,
                                        in_=etks[ci][:csz, 1, :, :])
                for ci, (t0, csz) in enumerate(chunks):
                    e0 = sb.tile([128, NH, 14], BF, tag=f"ec0{ci}{wi}",
                                 name=f"ec0{ci}{wi}")
                    src0 = bass.AP(
                        tensor=b0.tensor, offset=b0.offset + t0 * HREC + 13,
                        ap=[[HREC, csz], [27, NH], [1, 14]])
                    nc.sync.dma_start(out=e0[:csz, :, :], in_=src0)
                    e1 = sb.tile([128, NH, 14], BF, tag=f"ec1{ci}{wi}",
                                 name=f"ec1{ci}{wi}")
                    src1 = bass.AP(
                        tensor=b1.tensor, offset=b1.offset + t0 * 325 + 13,
                        ap=[[325, csz], [27, NH], [1, 14]])
                    nc.scalar.dma_start(out=e1[:csz, :, :], in_=src1)
                    ec[(0, ci, wi)] = e0
                    ec[(1, ci, wi)] = e1
                yield

            res.update(dict(xw=xw, qkvT=qkvT, ec=ec, chunks=chunks, rh=rh,
                            rw=rw, nreal=nreal, grp=grp, nfree=nfree))

        def stage2(t, wi, avT):
            """Attention for window wi of pair t -> avT[:, :, wi*TW...].

            Generator; yields between hp quanta."""
            rh = t["rh"]; rw = t["rw"]; nreal = t["nreal"]
            chunks = t["chunks"]; qkvT = t["qkvT"]; ec = t["ec"]
            woff = wi * TW
            cg = col_groups(rh, rw)
            for hp in range(6):
                # v^T tiles (both heads of the pair)
                vt = []
                for (j0, jn) in ((0, 128), (128, 68)):
                    pv = ps_at.tile([128, 2, 196], BF, tag="at", name="vtr")
                    nc.tensor.transpose(pv[:jn, 0, :128],
                                        qkvT[:, 12 + hp, woff + j0:woff + j0 + jn],
                                        ident)
                    vtk = sb.tile([128, 128], BF, tag=f"vtk{j0}")
                    nc.vector.tensor_copy(out=vtk[:jn, :], in_=pv[:jn, 0, :128])
                    vt.append((j0, jn, vtk))
                # scores for BOTH heads first, hh-inner so the two
                # row-group mms (lhsT base partition 0 vs 64) run
                # concurrently on the PE
                sps = [ps_at.tile([128, 2, 196], F32, tag="at", name="sp0"),
                       ps_at.tile([128, 2, 196], F32, tag="at", name="sp1")]
                for ci, (t0, csz) in enumerate(chunks):
                    for hh in (0, 1):
                        po = 64 * hh
                        nc.tensor.matmul(
                            sps[hh][:csz, ci, :],
                            qkvT[po:po + 64, hp, woff + t0:woff + t0 + csz],
                            qkvT[po:po + 64, 6 + hp, woff:woff + TW],
                            start=True, stop=True)
                yield
                ET = {}
                for hh in (0, 1):
                    h = 2 * hp + hh
                    sp = sps[hh]
                    E = sb.tile([128, 2, 196], BF, tag="E", name="E")
                    zt = sb2.tile([128, 4], F32, tag="z")
                    for ci, (t0, csz) in enumerate(chunks):
                        nc.scalar.activation(out=E[:csz, ci, :],
                                             in_=sp[:csz, ci, :],
                                             func=AF.Exp)
                        # row factor on the (otherwise idle) gpsimd engine
                        for (cst, nr2, r20, nc2, c20) in cg:
                            ev = E[:csz, ci, cst:cst + nr2 * nc2].rearrange(
                                "p (a b) -> p a b", a=nr2)
                            nc.gpsimd.tensor_mul(
                                out=ev, in0=ev,
                                in1=ec[(0, ci, wi)][:csz, h, r20:r20 + nr2]
                                [:, :, None].broadcast_to([csz, nr2, nc2]))
                        # col factor
                        for (cst, nr2, r20, nc2, c20) in cg:
                            ev = E[:csz, ci, cst:cst + nr2 * nc2].rearrange(
                                "p (a b) -> p a b", a=nr2)
                            nc.vector.tensor_mul(
                                out=ev, in0=ev,
                                in1=ec[(1, ci, wi)][:csz, h, c20:c20 + nc2]
                                [:, None, :].broadcast_to([csz, nr2, nc2]))
                        nc.vector.reduce_sum(out=zt[:csz, ci:ci + 1],
                                             in_=E[:csz, ci, :], axis=AX.X)
                    nch = len(chunks)
                    csz0 = chunks[0][1]
                    nc.vector.reciprocal(out=zt[:csz0, 2:2 + nch],
                                         in_=zt[:csz0, 0:nch])
                    ETt = [sb.tile([128, 196], BF, tag="ET0", name="ET0"),
                           sb.tile([128, 196], BF, tag="ET1", name="ET1")]
                    ET[hh] = ETt
                    for ci, (t0, csz) in enumerate(chunks):
                        nc.scalar.activation(
                            out=E[:csz, ci, :], in_=E[:csz, ci, :],
                            func=AF.Identity,
                            scale=zt[:csz, 2 + ci:3 + ci])
                        for ji, (j0, jn) in enumerate(((0, 128), (128, 68))):
                            pe = ps_at.tile([128, 2, 196], BF, tag="at",
                                            name="pe")
                            nc.tensor.transpose(pe[:jn, 0, :csz],
                                                E[:csz, ci, j0:j0 + jn],
                                                ident[:csz, :csz])
                            nc.vector.tensor_copy(
                                out=ETt[ji][:jn, t0:t0 + csz],
                                in_=pe[:jn, 0, :csz])
                    yield
                # AV, col-paired across hh
                avp = ps_av.tile([128, 196], F32, tag="av")
                for ji, (j0, jn, vtk) in enumerate(vt):
                    for hh in (0, 1):
                        nc.tensor.matmul(
                            avp[64 * hh:64 * hh + 64, :nreal],
                            vtk[:jn, 64 * hh:64 * hh + 64],
                            ET[hh][ji][:jn, :nreal],
                            start=(ji == 0), stop=(ji == 1),
                            tile_position=(0, 64 * hh))
                nc.vector.tensor_copy(out=avT[:, hp, woff:woff + nreal],
                                      in_=avp[:, :nreal])
                yield

        def proj_ln2(t, avT, offs, np_grp, out):
            """proj + residual + LN2 for the pair, packed cols. Generator."""
            grp = t["grp"]; xw = t["xw"]; nreal = t["nreal"]
            nw = len(grp)
            span = (nw - 1) * TW + nreal  # proj free span incl. slot gap
            ytile = sb1.tile([128, 6, 392], F32, tag="y", name="y")
            for oc in range(6):
                zp = ps_mm.tile([128, 512], F32, tag="mm")
                nc.tensor.matmul(zp[:, :span],
                                 bproj[0:1, 128 * oc:128 * (oc + 1)],
                                 ones_row[0:1, :span],
                                 start=True, stop=False)
                for kc in range(6):
                    nc.tensor.matmul(zp[:, :span],
                                     wproj[:, kc, 128 * oc:128 * (oc + 1)],
                                     avT[:, kc, :span],
                                     start=False, stop=(kc == 5))
                for wi in range(nw):
                    nc.vector.tensor_add(
                        out=ytile[:, oc, offs[wi]:offs[wi] + nreal],
                        in0=xw[:, oc, wi * TW:wi * TW + nreal],
                        in1=zp[:, wi * TW:wi * TW + nreal])
                if oc % 2 == 1:
                    yield

            # ---- LN2 on packed real cols ----
            rows2 = ln_rows(ytile, np_grp, True)
            yield
            ynb = sb1.tile([128, 6, 392], BF, tag="ynb", name="ynb")
            ln_apply(ytile, ynb, rows2, np_grp)
            yield
            out["ytile"] = ytile
            out["ynb"] = ynb

        def mlp(t, ytile, ynb, offs, np_grp):
            """MLP for the pair in two half-passes (halves hT footprint);
            half-A accumulates into ytile in place. Generator."""
            grp = t["grp"]; nreal = t["nreal"]
            for half in (0, 1):
                hT = sb1.tile([128, 12, 392], BF, tag="hT", name=f"hT{half}")
                for mi in range(12):
                    mc = 12 * half + mi
                    p1 = ps_mm.tile([128, 512], F32, tag="mm")
                    for kc in range(6):
                        nc.tensor.matmul(p1[:, :np_grp],
                                         w1[:, kc, 128 * mc:128 * (mc + 1)],
                                         ynb[:, kc, :np_grp],
                                         start=(kc == 0), stop=(kc == 5))
                    nc.scalar.activation(out=hT[:, mi, :np_grp],
                                         in_=p1[:, :np_grp],
                                         func=AF.Gelu, bias=b1c[:, mc:mc + 1])
                    if mi % 2 == 1:
                        yield
                for oc in range(6):
                    p2 = ps_mm.tile([128, 512], F32, tag="mm")
                    if half == 0:
                        nc.tensor.matmul(p2[:, :np_grp],
                                         b2r[0:1, 128 * oc:128 * (oc + 1)],
                                         ones_row[0:1, :np_grp],
                                         start=True, stop=False)
                    for ki in range(12):
                        kc = 12 * half + ki
                        nc.tensor.matmul(p2[:, :np_grp],
                                         w2[:, kc, 128 * oc:128 * (oc + 1)],
                                         hT[:, ki, :np_grp],
                                         start=(half == 1 and ki == 0),
                                         stop=(ki == 11))
                    if half == 0:
                        nc.vector.tensor_add(out=ytile[:, oc, :np_grp],
                                             in0=ytile[:, oc, :np_grp],
                                             in1=p2[:, :np_grp])
                    else:
                        fo = sb2.tile([128, 392], F32, tag="fo", bufs=1)
                        nc.vector.tensor_add(out=fo[:, :np_grp],
                                             in0=ytile[:, oc, :np_grp],
                                             in1=p2[:, :np_grp])
                        for wi, s in enumerate(grp):
                            nc.sync.dma_start(
                                out=out_d.ap()[128 * oc:128 * (oc + 1),
                                               s * TW:s * TW + nreal],
                                in_=fo[:, offs[wi]:offs[wi] + nreal])
                    yield

        def drain(*gens):
            """Round-robin the generators until all are exhausted."""
            live = [g for g in gens if g is not None]
            while live:
                nxt = []
                for g in live:
                    try:
                        next(g)
                        nxt.append(g)
                    except StopIteration:
                        pass
                live = nxt

        def attn_gen(t, avT, offs, np_grp, out):
            for wi in range(len(t["grp"])):
                yield from stage2(t, wi, avT)
            yield from proj_ln2(t, avT, offs, np_grp, out)

        tinfo = [dict() for _ in PAIRS]
        drain(stage1(0, tinfo[0]))
        mlp_gen = None
        for pi, grp in enumerate(PAIRS):
            t = tinfo[pi]
            nreal = t["nreal"]
            nw = len(grp)
            offs = [wi * nreal for wi in range(nw)]
            np_grp = nw * nreal
            avT = sb1.tile([128, 6, 392], BF, tag="avT", name="avT")
            out = {}
            s1_gen = stage1(pi + 1, tinfo[pi + 1]) \
                if pi + 1 < len(PAIRS) else None
            drain(attn_gen(t, avT, offs, np_grp, out), s1_gen, mlp_gen)
            mlp_gen = mlp(t, out["ytile"], out["ynb"], offs, np_grp)
        drain(mlp_gen)

    nc.compile()
    return nc


# ----------------------------------------------------------------------------
# host wrapper
# ----------------------------------------------------------------------------

def _window_assignment():
    interior = [(b, wy, wx) for b in range(B) for wy in range(4) for wx in range(4)]
    right = [(b, wy, 4) for b in range(B) for wy in range(4)]
    bottom = [(b, 4, wx) for b in range(B) for wx in range(4)]
    corner = [(b, 4, 4) for b in range(B)]
    cores = []
    for c in range(NCORES):
        wins = interior[8 * c:8 * c + 8] + right[2 * c:2 * c + 2] \
            + bottom[2 * c:2 * c + 2]
        wins.append(corner[c] if c < 4 else None)
        cores.append(wins)
    return cores


def _prep_consts(ln1_w, ln1_b, qkv_w, qkv_b, proj_w, proj_b, rel_pos_h,
                 rel_pos_w, ln2_w, ln2_b, w1, b1, w2, b2):
    qkv_w = np.asarray(qkv_w, np.float32)
    w1 = np.asarray(w1, np.float32)
    Wq = np.asarray(ln1_w, np.float32)[:, None] * qkv_w
    bq = np.asarray(qkv_b, np.float32) + np.asarray(ln1_b, np.float32) @ qkv_w
    Wq = Wq.copy()
    bq = bq.copy()
    Wq[:, :DIM] *= SCALE
    bq[:DIM] *= SCALE
    W1 = np.asarray(ln2_w, np.float32)[:, None] * w1
    B1 = np.asarray(b1, np.float32) + np.asarray(ln2_b, np.float32) @ w1
    return {
        "wqkv": Wq.reshape(6, 128, 3 * DIM).astype(BF16),
        "bqkv": np.ascontiguousarray(bq.reshape(18, 128).T).astype(np.float32),
        "relhw": np.ascontiguousarray(np.stack(
            [np.concatenate([np.asarray(t, np.float32)[::-1].T] * 2, axis=0)
             for t in (rel_pos_h, rel_pos_w)], axis=1)).astype(BF16),
        "wproj": np.asarray(proj_w, np.float32).reshape(6, 128, DIM).astype(BF16),
        "bproj": np.asarray(proj_b, np.float32).reshape(1, DIM).astype(BF16),
        "w1": W1.reshape(6, 128, MLP_H).astype(BF16),
        "b1": np.ascontiguousarray(B1.reshape(24, 128).T).astype(np.float32),
        "w2": np.asarray(w2, np.float32).reshape(24, 128, DIM).astype(BF16),
        "b2": np.asarray(b2, np.float32).reshape(1, DIM).astype(BF16),
    }


_ORDER_CACHE = {}


def _order_idx(rh, rw):
    key = (rh, rw)
    if key not in _ORDER_CACHE:
        _ORDER_CACHE[key] = np.array(token_order(rh, rw), np.int64)
    return _ORDER_CACHE[key]


def kernel(x, ln1_w, ln1_b, qkv_w, qkv_b, proj_w, proj_b, rel_pos_h,
           rel_pos_w, ln2_w, ln2_b, w1, b1, w2, b2):
    from concourse.bass_utils import run_bass_kernel_spmd

    x = np.asarray(x, np.float32)
    consts = _prep_consts(ln1_w, ln1_b, qkv_w, qkv_b, proj_w, proj_b,
                          rel_pos_h, rel_pos_w, ln2_w, ln2_b, w1, b1, w2, b2)

    if "nc" not in _CACHE:
        _CACHE["nc"] = _build()
    nc = _CACHE["nc"]

    assign = _window_assignment()
    xpad = np.zeros((B, 70, 70, DIM), np.float32)
    xpad[:, :H, :W, :] = x

    in_maps = []
    for c in range(NCORES):
        xwT = np.zeros((DIM, CORE_TOK), np.float32)
        for s, win in enumerate(assign[c]):
            if win is None:
                continue
            b, wy, wx = win
            rh, rw = WCLASSES[s]
            idx = _order_idx(rh, rw)
            blk = xpad[b, 14 * wy:14 * wy + 14, 14 * wx:14 * wx + 14, :]
            xwT[:, s * TW:(s + 1) * TW] = blk[idx[:, 0], idx[:, 1], :].T
        m = {"xwT": xwT.astype(BF16)}
        m.update(consts)
        in_maps.append(m)

    res = run_bass_kernel_spmd(nc, in_maps, core_ids=list(range(NCORES)),
                               **_CACHE.get("run_kwargs", {}))
    _CACHE["last_result"] = res

    out = np.zeros((B, H, W, DIM), np.float32)
    for c in range(NCORES):
        oT = res.results[c]["outT"]
        for s, win in enumerate(assign[c]):
            if win is None:
                continue
            b, wy, wx = win
            rh, rw = WCLASSES[s]
            idx = _order_idx(rh, rw)[:rh * rw]
            out[b, 14 * wy + idx[:, 0], 14 * wx + idx[:, 1], :] = \
                oT[:, s * TW:s * TW + rh * rw].T
    return out
